# revision 25
# baseline (speedup 1.0000x reference)
"""Trainium2 Bass kernel for nn_MACTitanLayer (MAC Titan layer, 8 cores).

Structure (v2):
  - Position-sharded encoder front (26 of 208 positions per core), bf16
    matmul operands, one activation table set (exp/tanh only; silu and
    sigmoid are built from tanh, inverse sqrt is Quake-init Newton on the
    vector engine).
  - K-sharded final matmul: core c owns contraction rows for its 26
    positions, streams its [12, 96, 26, 768] bf16 weight shard from HBM
    with a deep prefetch ring that starts at t=0.
  - The partial xf [768, 96] is combined with 4 chunked ReduceScatters
    (s-major row blocks) overlapped with the weight-stream matmuls; each
    core receives only its 96 tail tokens.
  - Token-sharded TTT tail (96 tokens/core): fused projection matmul
    (z|kp|vp|q2 in one 481-wide rhs), gradient partials via ones-column
    tricks, one small grad AllReduce, replicated param update, per-core
    retrieve; the host gathers the 8 output slices.
"""

import math

import numpy as np
import ml_dtypes

import concourse.bass as bass
import concourse.mybir as mybir
import concourse.tile as tile
from concourse import bacc
from concourse import bass_utils
from concourse.bass import ds
from concourse.masks import make_identity

F32 = mybir.dt.float32
BF16 = mybir.dt.bfloat16
I32 = mybir.dt.int32
AF = mybir.ActivationFunctionType
OP = mybir.AluOpType

B, S, H, PM, FF, NH = 8, 96, 96, 16, 2048, 2
ALPHA, THETA = 0.999, 0.3
L = PM + 2 * S            # 208 encoder tokens per batch
NC = 8
LSH = L // NC             # 26 positions per core
DK = LSH * H              # 2496 contraction rows per core
DOUT = S * H              # 9216
TQ = B * S                # 768 tokens
HD = H // NH              # 48
NTOK = B * L              # 1664
TSH = B * LSH             # 208 sharded tokens per core
CH = TQ // 2              # 384
OC = 768                  # big-matmul output chunk (8 s-positions)
NOC = DOUT // OC          # 12
NG = 4                    # ReduceScatter groups (3 chunks each)
TPC = TQ // NC            # 96 tail tokens per core
H2 = 2 * H                # 192
WCAT = H2 + (H + 1) + H + H  # 481: z | kp+ones | vp' | q2

CFG = {"w_bufs": 32, "ll2": 2}

_CACHE = {}

MAGIC = 0x5F3759DF


def _mm(nc, out, lhsT, rhs, start, stop):
    nc.tensor.matmul(out, lhsT, rhs, start=start, stop=stop)


def _rsqrt(nc, work, pool_tag, s_ap, out_ap, iters=2):
    """out = 1/sqrt(s) via Quake init + Newton, all on the vector engine.

    s_ap: f32 AP [P, T] (strictly positive). out_ap: f32 AP same shape.
    """
    shp = list(s_ap.shape)
    y = work.tile(shp, F32, tag=f"{pool_tag}_y", name="rs_y", bufs=1)
    t = work.tile(shp, F32, tag=f"{pool_tag}_t", name="rs_t", bufs=1)
    s = work.tile(shp, F32, tag=f"{pool_tag}_s", name="rs_s", bufs=1)
    nc.vector.tensor_copy(s[:], s_ap)
    # y0 bits = MAGIC - (bits(s) >> 1)  ==  ((bits(s)>>1) ^ -1) + (MAGIC+1)
    nc.vector.tensor_scalar(y[:].bitcast(I32), s[:].bitcast(I32),
                            1, None, OP.logical_shift_right)
    nc.vector.tensor_scalar(y[:].bitcast(I32), y[:].bitcast(I32),
                            -1, None, OP.bitwise_xor)
    nc.vector.tensor_scalar(y[:].bitcast(I32), y[:].bitcast(I32),
                            MAGIC + 1, None, OP.add)
    for it in range(iters):
        nc.vector.tensor_mul(t[:], s[:], y[:])
        nc.vector.tensor_mul(t[:], t[:], y[:])
        nc.vector.tensor_scalar(t[:], t[:], -0.5, 1.5, OP.mult, OP.add)
        nc.vector.tensor_mul(out_ap if it == iters - 1 else y[:], y[:], t[:])


def build(cfg):
    nc = bacc.Bacc("TRN2", target_bir_lowering=False, debug=False, num_devices=NC)

    def din(name, shape, dt=F32):
        return nc.dram_tensor(name, shape, dt, kind="ExternalInput")

    dd = dict(
        xT_d=din("xT", [H, TQ]),
        xTb_d=din("xTb", [H, TQ], BF16),
        pmT_d=din("pmT", [H, PM]),
        qwTb_d=din("qwTb", [H, H], BF16),
        qb_d=din("qb", [H, 1]),
        ipqT_d=din("ipqT", [H, NH, HD], BF16),
        ipkT_d=din("ipkT", [H, NH, HD], BF16),
        ipvT_d=din("ipvT", [H, H], BF16),
        ipqb_d=din("ipqb", [HD, NH, 1]),
        ipkb_d=din("ipkb", [HD, NH, 1]),
        ipvb_d=din("ipvb", [1, H]),
        opT_d=din("opT", [HD, NH, H], BF16),
        opb_d=din("opb", [H, 1]),
        ln1w_d=din("ln1w", [H, 1]), ln1b_d=din("ln1b", [H, 1]),
        ln2w_d=din("ln2w", [H, 1]), ln2b_d=din("ln2b", [H, 1]),
        f1T_d=din("f1T", [H, FF], BF16),
        f1b_d=din("f1b", [128, FF // 128, 1]),
        f1bh_d=din("f1bh", [128, FF // 128, 1]),
        f2T_d=din("f2T", [128, FF // 128, H], BF16),
        f2b_d=din("f2b", [H, 1]),
        m1Tb_d=din("m1Tb", [H + 1, H2], BF16),
        m1T_d=din("m1T", [H + 1, H2]),
        m1b_d=din("m1b", [H, 2, 1]),
        m2Tb_d=din("m2Tb", [H + 1, 2, H], BF16),
        m2T_d=din("m2T", [H + 1, 2, H]),
        m2b_d=din("m2b", [H, 1]),
        m2wb_d=din("m2wb", [H, H2], BF16),
        wcat_d=din("wcat", [H + 1, WCAT], BF16),
        fbs_d=din("fbs", [TPC, H]),
        wt_d=din("WTc", [NOC, 128, 20, OC], BF16),
    )
    dd["out_d"] = nc.dram_tensor("outf", [H, TQ], F32, kind="ExternalOutput")
    if cfg.get("debug"):
        for nm, shp in [("d_xcf", [H, B, L]), ("d_x1n", [H, TSH]),
                        ("d_xe", [H, TSH]), 
                        ("d_grads", [H + 1, 4 * H]),
                        ("d_q2T", [H, TPC]), ("d_nm1T", [H, H2]),
                        ("d_xff", [H, TPC]), ("d_kf", [HD, NH, B, L]),
                        ("d_h", [TPC, 194]), ("d_dz", [TPC, H2])]:
            dd[nm] = nc.dram_tensor(nm, shp, F32, kind="ExternalOutput")
        for nm, shp in [("d_arin", [TQ, H]), ("d_rs", [TQ, H])]:
            dd[nm] = nc.dram_tensor(nm, shp, BF16, kind="ExternalOutput")

    with tile.TileContext(nc) as tc:
        _body(nc, tc, dd, cfg)
    nc.compile()
    return nc


def _body(nc, tc, dd, cfg):
    from contextlib import ExitStack
    stack = ExitStack()

    def pool(name, bufs, space="SBUF"):
        return stack.enter_context(tc.tile_pool(name=name, bufs=bufs, space=space))

    const = pool("const", 1)
    big = pool("big", 1)
    work = pool("work", 1)
    wstr = pool("wstr", cfg["w_bufs"])
    psxp = pool("psxp", 2, "PSUM")
    pss = pool("pss", 3, "PSUM")
    pscp = pool("pscp", 1, "PSUM")
    dram = pool("dram", 1, "DRAM")
    ar_in = dram.tile([TQ, H], BF16, tag="ar_in", name="ar_in")
    ar_out = dram.tile([TQ, H], BF16, tag="ar_out", name="ar_out")
    gr_in = dram.tile([H + 1, 4 * H], BF16, tag="gr_in", name="gr_in")
    gr_out = dram.tile([H + 1, 4 * H], BF16, tag="gr_out", name="gr_out",
                       addr_space="Shared")

    def ld(dram_t, tag):
        t = const.tile(list(dram_t.shape), dram_t.dtype, tag=tag, name=tag)
        nc.sync.dma_start(t[:], dram_t[:])
        return t

    qwTb = ld(dd["qwTb_d"], "qwTb"); qb = ld(dd["qb_d"], "qb")
    ipqT = ld(dd["ipqT_d"], "ipqT"); ipkT = ld(dd["ipkT_d"], "ipkT")
    ipvT = ld(dd["ipvT_d"], "ipvT")
    ipqb = ld(dd["ipqb_d"], "ipqb"); ipkb = ld(dd["ipkb_d"], "ipkb")
    opT = ld(dd["opT_d"], "opT"); opb = ld(dd["opb_d"], "opb")
    ln1w = ld(dd["ln1w_d"], "ln1w"); ln1b = ld(dd["ln1b_d"], "ln1b")
    ln2w = ld(dd["ln2w_d"], "ln2w"); ln2b = ld(dd["ln2b_d"], "ln2b")
    f1T = ld(dd["f1T_d"], "f1T"); f1b = ld(dd["f1b_d"], "f1b")
    f1bh = ld(dd["f1bh_d"], "f1bh")
    f2T = ld(dd["f2T_d"], "f2T"); f2b = ld(dd["f2b_d"], "f2b")
    m1Tb = ld(dd["m1Tb_d"], "m1Tb"); m1T = ld(dd["m1T_d"], "m1T")
    m1b = ld(dd["m1b_d"], "m1b")
    m2Tb = ld(dd["m2Tb_d"], "m2Tb"); m2T = ld(dd["m2T_d"], "m2T")
    m2b = ld(dd["m2b_d"], "m2b")
    m2wb = ld(dd["m2wb_d"], "m2wb")
    wcat = ld(dd["wcat_d"], "wcat")
    fbs = ld(dd["fbs_d"], "fbs")
    pmT = ld(dd["pmT_d"], "pmT")

    vb_bc = const.tile([128, H], F32, tag="vb_bc", name="vb_bc")
    nc.sync.dma_start(vb_bc[:], dd["ipvb_d"][:].to_broadcast([128, H]))

    ident = const.tile([128, 128], F32, tag="ident", name="ident")
    make_identity(nc, ident[:])
    identb = const.tile([128, 128], BF16, tag="identb", name="identb")
    nc.vector.tensor_copy(identb[:], ident[:])
    ones_col = const.tile([128, 1], F32, tag="ones_col", name="ones_col")
    nc.vector.memset(ones_col[:], 1.0)
    ones_row = const.tile([1, H], F32, tag="ones_row", name="ones_row")
    nc.vector.memset(ones_row[:], 1.0)
    zb = const.tile([128, 1], F32, tag="zb", name="zb")
    nc.vector.memset(zb[:], 0.0)
    eps1 = const.tile([1, 1], F32, tag="eps1", name="eps1")
    nc.vector.memset(eps1[:], 1e-5)

    pid = nc.partition_id()
    qoff = pid * LSH

    # ============ F1: front ============
    xTb = big.tile([H, TQ], BF16, tag="xTb", name="xTb")
    nc.sync.dma_start(xTb[:], dd["xTb_d"][:])

    xcf = big.tile([H, B, L], F32, tag="xcf", name="xcf")
    xcb = big.tile([H, B, L], BF16, tag="xcb", name="xcb")
    nc.vector.tensor_copy(xcf[:, :, 0:PM],
                          pmT[:].unsqueeze(1).to_broadcast([H, B, PM]))
    nc.gpsimd.tensor_copy(xcb[:, :, 0:PM],
                          pmT[:].unsqueeze(1).to_broadcast([H, B, PM]))
    nc.sync.dma_start(xcf[:, :, PM + S:L],
                        dd["xT_d"][:].rearrange("h (b s) -> h b s", b=B))
    nc.gpsimd.tensor_copy(xcb[:, :, PM + S:L],
                          xTb[:].rearrange("h (b s) -> h b s", b=B))

    # neural-memory retrieve for all 768 tokens -> nmm region of xcf
    for c in range(2):
        sl = slice(c * CH, (c + 1) * CH)
        ps = pss.tile([H, CH], F32, tag="ps", name="ps_q1")
        _mm(nc, ps[:], qwTb[:], xTb[:, sl], True, True)
        q1c = work.tile([H, CH], F32, tag="q1c", name="q1c", bufs=2)
        nc.scalar.activation(q1c[:], ps[:], AF.Identity, bias=qb[:])
        sq = work.tile([H, CH], F32, tag="l2sq", name="l2sq", bufs=2)
        nc.vector.tensor_mul(sq[:], q1c[:], q1c[:])
        ps_s = pss.tile([1, CH], F32, tag="ps", name="ps_l2s")
        _mm(nc, ps_s[:], ones_col[:H, :], sq[:], True, True)
        rsq = work.tile([1, CH], F32, tag="l2r", name="l2r", bufs=1)
        _rsqrt(nc, work, "l2", ps_s[:], rsq[:])
        ps_b = pss.tile([H, CH], F32, tag="ps", name="ps_l2b")
        _mm(nc, ps_b[:], ones_row[:], rsq[:], True, True)
        qn = work.tile([H, CH], F32, tag="qn", name="qn", bufs=2)
        nc.vector.tensor_mul(qn[:], q1c[:], ps_b[:])
        qry = work.tile([H, CH], BF16, tag="qry", name="qry", bufs=2)
        nc.scalar.activation(qry[:], qn[:], AF.Silu, bias=zb[:H, :])
        h1 = []
        for m in range(2):
            psm = pss.tile([H, CH], F32, tag="ps", name="ps_h1")
            _mm(nc, psm[:], m1Tb[0:H, m * H:(m + 1) * H], qry[:], True, True)
            h1c = work.tile([H, CH], BF16, tag="h1c", name="h1c", bufs=2)
            nc.scalar.activation(h1c[:], psm[:], AF.Silu, bias=m1b[:, m, :])
            h1.append(h1c)
        ps2 = pss.tile([H, CH], F32, tag="ps", name="ps_nmm")
        _mm(nc, ps2[:], m2Tb[0:H, 0, :], h1[0][:], True, False)
        _mm(nc, ps2[:], m2Tb[0:H, 1, :], h1[1][:], False, True)
        nc.scalar.activation(
            xcf[:, c * 4:(c + 1) * 4, PM:PM + S],
            ps2[:].rearrange("h (b s) -> h b s", b=4), AF.Identity, bias=m2b[:])
        nc.gpsimd.tensor_copy(xcb[:, c * 4:(c + 1) * 4, PM:PM + S],
                              xcf[:, c * 4:(c + 1) * 4, PM:PM + S])

    # k (all tokens), q (own 26 positions)
    kf = big.tile([HD, NH, B, L], BF16, tag="kf", name="kf")
    q_sel = big.tile([HD, NH, B, LSH], BF16, tag="q_sel", name="q_sel")
    xcb_flat = xcb[:].rearrange("h b l -> h (b l)")
    ECH = NTOK // 4
    for hh in range(NH):
        for c in range(4):
            sl = slice(c * ECH, (c + 1) * ECH)
            ps = pss.tile([HD, ECH], F32, tag="ps", name="ps_k")
            _mm(nc, ps[:], ipkT[:, hh, :], xcb_flat[:, sl], True, True)
            nc.scalar.activation(
                kf[:].rearrange("d n b l -> d n (b l)")[:, hh, sl],
                ps[:], AF.Identity, bias=ipkb[:, hh, :])
        psq = pss.tile([HD, TSH], F32, tag="ps", name="ps_q")
        _mm(nc, psq[:], ipqT[:, hh, :], xcb[:, :, ds(qoff, LSH)], True, True)
        nc.scalar.activation(q_sel[:, hh, :, :],
                             psq[:].rearrange("d (b l) -> d b l", b=B),
                             AF.Identity, bias=ipqb[:, hh, :])

    # v token-major per batch: [128, B, H] + [80, B, H] (bf16)
    v_tm0 = big.tile([128, B, H], BF16, tag="v_tm0", name="v_tm0")
    v_tm1 = big.tile([80, B, H], BF16, tag="v_tm1", name="v_tm1")
    for b in range(B):
        for tt, dst, npart in ((0, v_tm0, 128), (1, v_tm1, 80)):
            ps = pss.tile([128, H], F32, tag="ps", name="ps_v")
            toks = slice(tt * 128, tt * 128 + npart)
            _mm(nc, ps[:npart, :], xcb[:, b, toks], ipvT[:], True, True)
            nc.vector.tensor_add(dst[:, b, :], ps[:npart, :], vb_bc[:npart, :])

    # attention: scores/exp batched over pairs of batches per head
    of = big.tile([HD, NH, B, LSH], BF16, tag="of", name="of")
    for hh in range(NH):
        for bg in range(4):
            sc2 = pss.tile([64, L], F32, tag="ps", name="sc2")
            for bq in range(2):
                b = bg * 2 + bq
                _mm(nc, sc2[bq * 32:bq * 32 + LSH, :],
                    q_sel[:, hh, b, :], kf[:, hh, b, :], True, True)
            e2a = work.tile([64, L], BF16, tag="e2a", name="e2a", bufs=2)
            den2 = work.tile([64, 1], F32, tag="den2", name="den2", bufs=2)
            nc.scalar.activation(e2a[:], sc2[:], AF.Exp, bias=zb[:64, :],
                                 accum_out=den2[:])
            rden = work.tile([64, 1], F32, tag="rden", name="rden", bufs=2)
            nc.vector.reciprocal(rden[:], den2[:])
            a2 = work.tile([64, L], BF16, tag="a2", name="a2", bufs=2)
            nc.vector.tensor_scalar_mul(a2[:], e2a[:], rden[:])
            at2 = work.tile([128, 2, 64], BF16, tag="at2", name="at2", bufs=2)
            for kc, npart in ((0, 128), (1, 80)):
                pst = pss.tile([128, 64], BF16, tag="ps", name="ps_t")
                nc.tensor.transpose(pst[:npart, :],
                                    a2[:, kc * 128:kc * 128 + npart],
                                    identb[:64, :64])
                nc.vector.tensor_copy(at2[:npart, kc, :], pst[:npart, :])
            for bq in range(2):
                b = bg * 2 + bq
                ps_o = pss.tile([HD, LSH], F32, tag="ps", name="ps_o")
                for tt, vsrc, npart in ((0, v_tm0, 128), (1, v_tm1, 80)):
                    _mm(nc, ps_o[:], vsrc[:, b, hh * HD:(hh + 1) * HD],
                        at2[:npart, tt, bq * 32:bq * 32 + LSH], tt == 0, tt == 1)
                nc.scalar.copy(of[:, hh, b, :], ps_o[:])

    # out_proj + residual
    ps_op = pss.tile([H, TSH], F32, tag="ps", name="ps_op")
    for hh in range(NH):
        _mm(nc, ps_op[:], opT[:, hh, :],
            of[:, hh, :, :].rearrange("d b l -> d (b l)"), hh == 0, hh == 1)
    x1 = big.tile([H, TSH], F32, tag="x1", name="x1")
    nc.vector.tensor_scalar_add(x1[:], ps_op[:], opb[:])
    nc.vector.tensor_add(x1[:].rearrange("h (b l) -> h b l", b=B),
                         x1[:].rearrange("h (b l) -> h b l", b=B),
                         xcf[:, :, ds(qoff, LSH)])

    x1n = big.tile([H, TSH], F32, tag="x1n", name="x1n")
    _layernorm(nc, pss, work, x1[:], x1n[:], ln1w, ln1b, ones_col, ones_row, eps1)
    x1nb = big.tile([H, TSH], BF16, tag="x1nb", name="x1nb")
    nc.gpsimd.tensor_copy(x1nb[:], x1n[:])

    ps2f = pss.tile([H, TSH], F32, tag="ps", name="ps_ff2")
    for m in range(FF // 128):
        psf = pss.tile([128, TSH], F32, tag="ps", name="ps_ff1")
        _mm(nc, psf[:], f1T[:, m * 128:(m + 1) * 128], x1nb[:], True, True)
        h_ffn = work.tile([128, TSH], BF16, tag="h_ffn", name="h_ffn", bufs=3)
        nc.scalar.activation(h_ffn[:], psf[:], AF.Silu, bias=f1b[:, m, :])
        _mm(nc, ps2f[:], f2T[:, m, :], h_ffn[:], m == 0, m == FF // 128 - 1)
    x2 = big.tile([H, TSH], F32, tag="x2", name="x2")
    nc.vector.tensor_scalar_add(x2[:], ps2f[:], f2b[:])
    nc.vector.tensor_add(x2[:], x2[:], x1n[:])

    e2 = big.tile([H, TSH], F32, tag="e2", name="e2")
    _layernorm(nc, pss, work, x2[:], e2[:], ln2w, ln2b, ones_col, ones_row, eps1)
    xeb = big.tile([H, TSH], BF16, tag="xeb", name="xeb")
    nc.scalar.activation(xeb[:], e2[:], AF.Silu, bias=zb[:H, :])
    # repack xe to K=128 lhsT tiles [128, 20, 8] via a DRAM round-trip
    xe_dram = dram.tile([1, 8 * DK + 128], F32, tag="xe_dram", name="xe_dram")
    xe_tok0 = big.tile([128, H], F32, tag="xe_tok0", name="xe_tok0")
    xe_tok1 = big.tile([80, H], F32, tag="xe_tok1", name="xe_tok1")
    for tt, dst, npart in ((0, xe_tok0, 128), (1, xe_tok1, 80)):
        pst = pss.tile([128, H], F32, tag="ps", name="ps_xet")
        nc.tensor.transpose(pst[:npart, :], e2[:, tt * 128:tt * 128 + npart],
                            ident[:H, :H])
        nc.scalar.activation(dst[:], pst[:npart, :], AF.Silu, bias=zb[:npart, :])
    nc.scalar.dma_start(
        xe_dram[0, 0:128 * H].rearrange("(t h) -> t h", h=H), xe_tok0[:])
    nc.scalar.dma_start(
        xe_dram[0, 128 * H:TSH * H].rearrange("(t h) -> t h", h=H), xe_tok1[:])
    xe128f = big.tile([128, 20, B], F32, tag="xe128f", name="xe128f")
    for b in range(B):
        eng = nc.scalar if b % 2 == 0 else nc.gpsimd
        eng.dma_start(
            xe128f[:, :, b],
            xe_dram[0, b * DK:b * DK + 2560].rearrange("(c p) -> p c", p=128))
    xe128 = big.tile([128, 20, B], BF16, tag="xe128", name="xe128")
    nc.vector.tensor_copy(xe128[0:64, :, :], xe128f[0:64, :, :])
    nc.vector.tensor_copy(xe128[64:128, 0:19, :], xe128f[64:128, 0:19, :])
    nc.vector.memset(xe128[64:128, 19, :], 0.0)

    if cfg.get("debug"):
        nc.sync.dma_start(dd["d_xcf"][:], xcf[:])
        nc.sync.dma_start(dd["d_x1n"][:], x1n[:])
        dxe = work.tile([H, TSH], F32, tag="dxe", name="dxe")
        nc.vector.tensor_copy(dxe[:], xeb[:])
        nc.sync.dma_start(dd["d_xe"][:], dxe[:])
        dkf = work.tile([HD, NH, B, L], F32, tag="dkf", name="dkf")
        nc.vector.tensor_copy(dkf[:], kf[:])
        nc.sync.dma_start(dd["d_kf"][:], dkf[:])

    # ============ F2: big matmul (K-sharded) + chunked ReduceScatter ============
    # ar_in rows are (s, b)-major; the chunk DMA writes through a
    # batch-first view so src (SBUF partition=b) and dst dims align.
    ar_in_b = ar_in[:].rearrange("(s b) h -> b s h", b=B)   # [8, 96, 96] strided
    wt4 = dd["wt_d"][:]
    for ci in range(NOC):
        psx = psxp.tile([B, OC], F32, tag="psx", name="psx")
        for k0 in range(0, 20, 2):
            wt = wstr.tile([128, 2, OC], BF16, tag="wt", name="wt")
            nc.sync.dma_start(wt[:], wt4[ci, :, k0:k0 + 2, :])
            for k1 in range(2):
                kg = k0 + k1
                for j0 in range(0, OC, 512):
                    j1 = min(j0 + 512, OC)
                    _mm(nc, psx[:, j0:j1], xe128[:, kg, :], wt[:, k1, j0:j1],
                        kg == 0, kg == 19)
        xfp = work.tile([B, OC], BF16, tag="xfp", name="xfp", bufs=2)
        nc.scalar.copy(xfp[:], psx[:])
        nc.scalar.dma_start(ar_in_b[:, 8 * ci:8 * ci + 8, :],
                            xfp[:].rearrange("b (s h) -> b s h", s=8))
    nc.gpsimd.collective_compute(
        "AllReduce", OP.add,
        replica_groups=[list(range(NC))],
        ins=[ar_in[:, :].opt()],
        outs=[ar_out[:, :].opt()],
    )

    # ============ F3: tail (96 tokens per core) ============
    xf_bf = big.tile([TPC, H], BF16, tag="xf_bf", name="xf_bf")
    for g in range(NG):
        nc.scalar.dma_start(xf_bf[24 * g:24 * g + 24, :],
                            ar_out[ds(192 * g + pid * 24, 24), :])
    xf_tm = big.tile([TPC, H], F32, tag="xf_tm", name="xf_tm")
    nc.vector.tensor_add(xf_tm[:], xf_bf[:], fbs[:])

    ps_xt = pss.tile([H, TPC], F32, tag="ps", name="ps_xt")
    nc.tensor.transpose(ps_xt[:], xf_tm[:], ident[:TPC, :TPC])
    xffT = big.tile([H, TPC], F32, tag="xffT", name="xffT")
    nc.vector.tensor_copy(xffT[:], ps_xt[:])
    xffTb = big.tile([H + 1, TPC], BF16, tag="xffTb", name="xffTb")
    nc.vector.tensor_copy(xffTb[0:H, :], ps_xt[:])
    nc.vector.memset(xffTb[H:H + 1, :], 1.0)

    psc = pscp.tile([TPC, WCAT], F32, tag="psc", name="psc")
    _mm(nc, psc[:], xffTb[:], wcat[:], True, True)
    zsl = psc[:, 0:H2]
    kpsl = psc[:, H2:H2 + H + 1]
    vpsl = psc[:, H2 + H + 1:H2 + 2 * H + 1]
    q2sl = psc[:, H2 + 2 * H + 1:WCAT]

    kp_b = big.tile([TPC, H + 1], BF16, tag="kp_b", name="kp_b")
    nc.vector.tensor_copy(kp_b[:], kpsl)
    q2r = big.tile([TPC, H], F32, tag="q2r", name="q2r")
    nc.vector.tensor_copy(q2r[:], q2sl)
    vp_sb = big.tile([TPC, H], F32, tag="vp_sb", name="vp_sb")
    nc.vector.tensor_copy(vp_sb[:], vpsl)

    # h = silu(z), sp = sig(z)*(1 + z - h)   (z includes fused bias)
    thz = work.tile([TPC, H2], F32, tag="thz", name="thz")
    nc.scalar.activation(thz[:], zsl, AF.Tanh, bias=zb[:TPC, :], scale=0.5)
    nc.vector.tensor_scalar(thz[:], thz[:], 0.5, 0.5, OP.mult, OP.add)
    h_tm = big.tile([TPC, 194], BF16, tag="h_tm", name="h_tm")
    nc.vector.tensor_mul(h_tm[:, 0:H], psc[:, 0:H], thz[:, 0:H])
    nc.vector.tensor_mul(h_tm[:, H + 1:2 * H + 1], psc[:, H:H2], thz[:, H:H2])
    nc.vector.memset(h_tm[:, H:H + 1], 1.0)
    nc.vector.memset(h_tm[:, 2 * H + 1:2 * H + 2], 1.0)
    sp = big.tile([TPC, H2], F32, tag="sp", name="sp")
    nc.vector.tensor_scalar(sp[:], thz[:], -1.0, 1.0, OP.mult, OP.add)
    nc.vector.tensor_mul(sp[:], sp[:], zsl)
    nc.vector.tensor_scalar_add(sp[:], sp[:], 1.0)
    nc.vector.tensor_mul(sp[:], sp[:], thz[:])

    # pred via h_fm
    h_fm = big.tile([H, 2, TPC], BF16, tag="h_fm", name="h_fm")
    for m in range(2):
        pst = pss.tile([H, TPC], BF16, tag="ps", name="ps_hf")
        nc.tensor.transpose(pst[:], h_tm[:, m * (H + 1):m * (H + 1) + H],
                            identb[:TPC, :TPC])
        nc.vector.tensor_copy(h_fm[:, m, :], pst[:])
    psp = pss.tile([TPC, H], F32, tag="ps", name="ps_pred")
    _mm(nc, psp[:], h_fm[:, 0, :], m2Tb[0:H, 0, :], True, False)
    _mm(nc, psp[:], h_fm[:, 1, :], m2Tb[0:H, 1, :], False, True)
    dpr_b = big.tile([TPC, H], BF16, tag="dpr_b", name="dpr_b")
    nc.vector.tensor_sub(dpr_b[:], psp[:], vp_sb[:])

    pst2 = pss.tile([H, TPC], BF16, tag="ps", name="ps_dprT")
    nc.tensor.transpose(pst2[:], dpr_b[:], identb[:TPC, :TPC])
    dprT = big.tile([H, TPC], BF16, tag="dprT", name="dprT")
    nc.vector.tensor_copy(dprT[:], pst2[:])

    psd = pss.tile([TPC, H2], F32, tag="ps", name="ps_dz")
    _mm(nc, psd[:], dprT[:], m2wb[:], True, True)
    dz_b = big.tile([TPC, H2], BF16, tag="dz_b", name="dz_b")
    nc.vector.tensor_mul(dz_b[:], psd[:], sp[:])

    # grads: g1 [97, 192] = kp_aug^T dz ; g2 [97, 2, 96] = h_aug^T dpr
    psg1 = pss.tile([H + 1, H2], F32, tag="ps", name="ps_g1")
    _mm(nc, psg1[:], kp_b[:], dz_b[:], True, True)
    psg2 = pss.tile([H + 1, 2, H], F32, tag="ps", name="ps_g2")
    for m in range(2):
        _mm(nc, psg2[:, m, :], h_tm[:, m * (H + 1):(m + 1) * (H + 1)],
            dpr_b[:], True, True)
    grads = big.tile([H + 1, 4 * H], BF16, tag="grads", name="grads")
    nc.vector.tensor_copy(grads[:, 0:H2], psg1[:])
    nc.vector.tensor_copy(grads[:, H2:4 * H],
                          psg2[:].rearrange("p m h -> p (m h)"))
    nc.scalar.dma_start(gr_in[:, :], grads[:])
    nc.gpsimd.collective_compute(
        "AllReduce", OP.add,
        replica_groups=[list(range(NC))],
        ins=[gr_in[:, :].opt()],
        outs=[gr_out[:, :].opt()],
    )

    # q2 = l2norm over features (token-major) then transpose
    sqq = work.tile([TPC, H], F32, tag="sqq", name="sqq")
    nc.vector.tensor_mul(sqq[:], q2r[:], q2r[:])
    ssum = work.tile([TPC, 1], F32, tag="ssum", name="ssum")
    nc.vector.reduce_sum(ssum[:], sqq[:], axis=mybir.AxisListType.X)
    rs2 = work.tile([TPC, 1], F32, tag="rs2", name="rs2")
    _rsqrt(nc, work, "l2t", ssum[:], rs2[:])
    q2n = work.tile([TPC, H], BF16, tag="q2n", name="q2n")
    nc.vector.tensor_scalar_mul(q2n[:], q2r[:], rs2[:])
    ps_qt = pss.tile([H, TPC], BF16, tag="ps", name="ps_q2T")
    nc.tensor.transpose(ps_qt[:], q2n[:], identb[:TPC, :TPC])
    q2T = big.tile([H + 1, TPC], BF16, tag="q2T", name="q2T")
    nc.vector.tensor_copy(q2T[0:H, :], ps_qt[:])
    nc.vector.memset(q2T[H:H + 1, :], 1.0)

    # ============ F4: finalize ============
    grd = big.tile([H + 1, 4 * H], BF16, tag="grd", name="grd")
    nc.scalar.dma_start(grd[:], gr_out[:, :])
    THP = THETA * 2.0 / (TQ * H)

    if cfg.get("debug"):
        nc.sync.dma_start(dd["d_arin"][:], ar_in[:, :])
        nc.sync.dma_start(dd["d_rs"][:], ar_out[:, :])
        nc.sync.dma_start(dd["d_grads"][:], gr_out[:, :])
        nc.sync.dma_start(dd["d_xff"][:], xffT[:])
        dq2 = work.tile([H, TPC], F32, tag="dq2", name="dq2")
        nc.vector.tensor_copy(dq2[:], q2T[:])
        nc.sync.dma_start(dd["d_q2T"][:], dq2[:])
        dh = work.tile([TPC, 194], F32, tag="dh", name="dh")
        nc.vector.tensor_copy(dh[:], h_tm[:])
        nc.sync.dma_start(dd["d_h"][:], dh[:])
        ddz = work.tile([TPC, H2], F32, tag="ddz", name="ddz")
        nc.vector.tensor_copy(ddz[:], dz_b[:])
        nc.sync.dma_start(dd["d_dz"][:], ddz[:])

    nm1T = big.tile([H + 1, H2], F32, tag="nm1T", name="nm1T")
    tg1 = work.tile([H + 1, H2], F32, tag="tg1", name="tg1")
    nc.vector.tensor_scalar_mul(tg1[:], grd[:, 0:H2], THP)
    nc.vector.tensor_scalar_mul(nm1T[:], m1T[:], ALPHA)
    nc.vector.tensor_sub(nm1T[:], nm1T[:], tg1[:])
    nm1Tb = big.tile([H + 1, H2], BF16, tag="nm1Tb", name="nm1Tb")
    nc.vector.tensor_copy(nm1Tb[:], nm1T[:])
    if cfg.get("debug"):
        nc.sync.dma_start(dd["d_nm1T"][:], nm1T[0:H, :])

    nm2T = big.tile([H + 1, 2, H], BF16, tag="nm2T", name="nm2T")
    tg2 = work.tile([H + 1, 2, H], F32, tag="tg2", name="tg2")
    nc.vector.tensor_scalar_mul(tg2[:].rearrange("p m h -> p (m h)"),
                                grd[:, H2:4 * H], THP)
    tg2b = work.tile([H + 1, 2, H], F32, tag="tg2b", name="tg2b")
    nc.vector.tensor_scalar_mul(tg2b[:].rearrange("p m h -> p (m h)"),
                                m2T[:].rearrange("p m h -> p (m h)"), ALPHA)
    nc.vector.tensor_sub(nm2T[:].rearrange("p m h -> p (m h)"),
                         tg2b[:].rearrange("p m h -> p (m h)"),
                         tg2[:].rearrange("p m h -> p (m h)"))
    # chunk-1 bias row must not double-count nm2b
    nc.vector.memset(nm2T[H:H + 1, 1, :], 0.0)

    # retrieve with updated memory for own 96 tokens (biases ride the
    # augmented ones rows)
    uu = []
    for m in range(2):
        psu = pss.tile([H, TPC], F32, tag="ps", name="ps_u")
        _mm(nc, psu[:], nm1Tb[:, m * H:(m + 1) * H], q2T[:], True, True)
        thu = work.tile([H, TPC], F32, tag="thu", name="thu", bufs=2)
        nc.scalar.activation(thu[:], psu[:], AF.Tanh, bias=zb[:H, :], scale=0.5)
        nc.vector.tensor_scalar(thu[:], thu[:], 0.5, 0.5, OP.mult, OP.add)
        u_b = work.tile([H + 1, TPC], BF16, tag="u_b", name="u_b", bufs=2)
        nc.vector.tensor_mul(u_b[0:H, :], psu[:], thu[:])
        nc.vector.memset(u_b[H:H + 1, :], 1.0)
        uu.append(u_b)
    psy = pss.tile([H, TPC], F32, tag="ps", name="ps_y")
    _mm(nc, psy[:], nm2T[:, 0, :], uu[0][:], True, False)
    _mm(nc, psy[:], nm2T[:, 1, :], uu[1][:], False, True)
    thy = work.tile([H, TPC], F32, tag="thy", name="thy")
    nc.scalar.activation(thy[:], psy[:], AF.Tanh, bias=zb[:H, :], scale=0.5)
    nc.vector.tensor_scalar(thy[:], thy[:], 0.5, 0.5, OP.mult, OP.add)
    ot = work.tile([H, TPC], F32, tag="ot", name="ot")
    nc.vector.tensor_mul(ot[:], xffT[:], thy[:])

    out3 = dd["out_d"][:].rearrange("h (b s) -> h b s", b=B)
    ot_b = work.tile([H, B, NG * 3], F32, tag="ot_b", name="ot_b")
    nc.vector.tensor_copy(ot_b[:],
                          ot[:].rearrange("h (g sp b) -> h b (g sp)", g=NG, sp=3))
    for g in range(NG):
        nc.scalar.dma_start(
            out3[:, :, ds(24 * g + pid * 3, 3)],
            ot_b[:, :, 3 * g:3 * g + 3])

    stack.close()


def _layernorm(nc, pss, work, src_ap, dst_ap, w_ap, b_ap, ones_col, ones_row, eps1):
    """dst = LN(src) * w + b over the feature (partition) axis; [96, T] APs."""
    T = src_ap.shape[-1]
    ps_s = pss.tile([1, T], F32, tag="ps", name="ps_lns")
    _mm(nc, ps_s[:], ones_col[:H, :], src_ap, True, True)
    mean = work.tile([1, T], F32, tag="ln_mean", name="ln_mean")
    nc.vector.tensor_scalar_mul(mean[:], ps_s[:], 1.0 / H)
    sq = work.tile([H, T], F32, tag="ln_sq", name="ln_sq")
    nc.vector.tensor_mul(sq[:], src_ap, src_ap)
    ps_q = pss.tile([1, T], F32, tag="ps", name="ps_lnq")
    _mm(nc, ps_q[:], ones_col[:H, :], sq[:], True, True)
    var = work.tile([1, T], F32, tag="ln_var", name="ln_var")
    nc.scalar.activation(var[:], ps_q[:], AF.Identity, bias=eps1[:], scale=1.0 / H)
    m2t = work.tile([1, T], F32, tag="ln_m2", name="ln_m2")
    nc.vector.tensor_mul(m2t[:], mean[:], mean[:])
    nc.vector.tensor_sub(var[:], var[:], m2t[:])
    rstd = work.tile([1, T], F32, tag="ln_rstd", name="ln_rstd")
    _rsqrt(nc, work, "ln", var[:], rstd[:])
    nmr = work.tile([1, T], F32, tag="ln_nmr", name="ln_nmr")
    nc.vector.tensor_mul(nmr[:], mean[:], rstd[:])
    nc.vector.tensor_scalar_mul(nmr[:], nmr[:], -1.0)
    ps_a = pss.tile([H, T], F32, tag="ps", name="ps_lna")
    _mm(nc, ps_a[:], ones_row[:], rstd[:], True, True)
    ps_c = pss.tile([H, T], F32, tag="ps", name="ps_lnc")
    _mm(nc, ps_c[:], ones_row[:], nmr[:], True, True)
    t1 = work.tile([H, T], F32, tag="ln_t1", name="ln_t1")
    nc.vector.tensor_mul(t1[:], src_ap, ps_a[:])
    nc.vector.tensor_add(t1[:], t1[:], ps_c[:])
    nc.vector.tensor_scalar(dst_ap, t1[:], w_ap[:], b_ap[:], OP.mult, OP.add)


def prep_inmaps(inputs, cfg=None):
    cfg = cfg or CFG
    f32 = np.float32
    bf16 = ml_dtypes.bfloat16

    def T(a):
        return np.ascontiguousarray(np.asarray(a, f32).T)

    x = np.asarray(inputs["x"], f32)
    ipw = np.asarray(inputs["in_proj_w"], f32)   # [288, 96]
    ipb = np.asarray(inputs["in_proj_b"], f32)   # [288]
    sc = 1.0 / math.sqrt(HD)
    qw_part = ipw[0:H] * sc
    qb_part = ipb[0:H] * sc
    kw_part = ipw[H:2 * H]
    kb_part = ipb[H:2 * H]
    vw_part = ipw[2 * H:3 * H]
    vb_part = ipb[2 * H:3 * H]

    ipqT = np.ascontiguousarray(qw_part.T.reshape(H, NH, HD))
    ipkT = np.ascontiguousarray(kw_part.T.reshape(H, NH, HD))
    ipqb = np.ascontiguousarray(qb_part.reshape(NH, HD).T.reshape(HD, NH, 1))
    ipkb = np.ascontiguousarray(kb_part.reshape(NH, HD).T.reshape(HD, NH, 1))

    opw = np.asarray(inputs["out_proj_w"], f32)
    opT = np.ascontiguousarray(opw.T.reshape(NH, HD, H).transpose(1, 0, 2))

    f1b = np.asarray(inputs["ff1_b"], f32).reshape(FF // 128, 128, 1)
    f1b = np.ascontiguousarray(f1b.transpose(1, 0, 2))
    f2T = T(inputs["ff2_w"])
    f2T = np.ascontiguousarray(f2T.reshape(FF // 128, 128, H).transpose(1, 0, 2))

    m1w = np.asarray(inputs["m1_w"], f32)        # [192, 96]
    m1bv = np.asarray(inputs["m1_b"], f32)       # [192]
    m2w = np.asarray(inputs["m2_w"], f32)        # [96, 192]
    m2bv = np.asarray(inputs["m2_b"], f32)       # [96]
    kw = np.asarray(inputs["k_w"], f32)
    kb = np.asarray(inputs["k_b"], f32)
    vw = np.asarray(inputs["v_w"], f32)
    vb = np.asarray(inputs["v_b"], f32)
    qw = np.asarray(inputs["q_w"], f32)
    qbv = np.asarray(inputs["q_b"], f32)

    m1b = np.ascontiguousarray(m1bv.reshape(2, H, 1).transpose(1, 0, 2))
    m1T_aug = np.concatenate([T(m1w), m1bv.reshape(1, H2)], 0)        # [97, 192]
    m2T3 = np.ascontiguousarray(T(m2w).reshape(2, H, H).transpose(1, 0, 2))
    m2T_aug = np.zeros((H + 1, 2, H), f32)
    m2T_aug[0:H] = m2T3
    m2T_aug[H, 0] = m2bv                                              # chunk-0 bias row

    # Wcat [97, 481]: z | kp(+ones col) | vp' | q2
    wcat = np.zeros((H + 1, WCAT), f32)
    m1kw = m1w @ kw                               # [192, 96]
    wcat[0:H, 0:H2] = m1kw.T
    wcat[H, 0:H2] = m1w @ kb + m1bv
    wcat[0:H, H2:H2 + H] = kw.T
    wcat[H, H2:H2 + H] = kb
    wcat[H, H2 + H] = 1.0                         # ones column for g1 bias row
    wcat[0:H, H2 + H + 1:H2 + 2 * H + 1] = vw.T
    wcat[H, H2 + H + 1:H2 + 2 * H + 1] = vb - m2bv
    wcat[0:H, H2 + 2 * H + 1:WCAT] = qw.T
    wcat[H, H2 + 2 * H + 1:WCAT] = qbv

    fwT = np.ascontiguousarray(np.asarray(inputs["final_w"], f32).T)
    fb = np.asarray(inputs["final_b"], f32).reshape(S, H)

    col = lambda k: np.ascontiguousarray(np.asarray(inputs[k], f32).reshape(-1, 1))
    xTf = T(x.reshape(TQ, H))
    base = dict(
        xT=xTf, xTb=xTf.astype(bf16),
        pmT=T(inputs["persistent_memory"]),
        qwTb=T(qw).astype(bf16), qb=col("q_b"),
        ipqT=ipqT.astype(bf16), ipkT=ipkT.astype(bf16),
        ipvT=np.ascontiguousarray(vw_part.T).astype(bf16),
        ipqb=ipqb, ipkb=ipkb,
        ipvb=np.ascontiguousarray(vb_part.reshape(1, H)),
        opT=opT.astype(bf16), opb=col("out_proj_b"),
        ln1w=col("ln1_w"), ln1b=col("ln1_b"),
        ln2w=col("ln2_w"), ln2b=col("ln2_b"),
        f1T=T(inputs["ff1_w"]).astype(bf16), f1b=f1b,
        f1bh=np.ascontiguousarray(f1b * 0.5),
        f2T=f2T.astype(bf16), f2b=col("ff2_b"),
        m1Tb=m1T_aug.astype(bf16), m1T=m1T_aug,
        m1b=m1b,
        m2Tb=m2T_aug.astype(bf16), m2T=m2T_aug,
        m2b=col("m2_b"),
        m2wb=np.ascontiguousarray(m2w).astype(bf16),
        wcat=wcat.astype(bf16),
    )
    in_maps = []
    for c in range(NC):
        m = dict(base)
        shard = fwT[c * DK:(c + 1) * DK]                     # [(l h), 9216]
        shard_pad = np.concatenate([shard, np.zeros((64, DOUT), f32)], 0)
        packed = shard_pad.reshape(20, 128, NOC, OC).transpose(2, 1, 0, 3)
        m["WTc"] = np.ascontiguousarray(packed.astype(bf16))
        # fbs: final_b rows for this core's tokens in (g, sp, b) order
        ss = np.array([24 * g + 3 * c + d for g in range(NG) for d in range(3)])
        m["fbs"] = np.ascontiguousarray(
            np.repeat(fb[ss], B, axis=0))                    # [96, 96]
        in_maps.append(m)
    return in_maps


def get_nc(cfg=None):
    cfg = cfg or CFG
    key = tuple(sorted((k, str(v)) for k, v in cfg.items()))
    if key not in _CACHE:
        _CACHE[key] = build(cfg)
    return _CACHE[key]


def assemble(results):
    """Gather per-core output column slices into the full [B, S, H] output."""
    full = np.zeros((H, TQ), np.float32)
    for c in range(NC):
        outc = results[c]["outf"]                            # [96, 768]
        cols = np.array([b * S + 24 * g + 3 * c + d
                         for g in range(NG) for d in range(3) for b in range(B)])
        full[:, cols] = outc[:, cols]
    return np.ascontiguousarray(full.T).reshape(B, S, H)


def kernel(**inputs):
    nc = get_nc()
    in_maps = prep_inmaps(inputs)
    res = bass_utils.run_bass_kernel_spmd(
        nc, in_maps, core_ids=list(range(NC)), trace=False
    )
    return assemble(res.results)


if __name__ == "__main__":
    print("building...")
    get_nc()
    print("built")


# revision 26
# speedup vs baseline: 1.0352x; 1.0352x over previous
"""Trainium2 Bass kernel for nn_MACTitanLayer (MAC Titan layer, 8 cores).

Structure (v2):
  - Position-sharded encoder front (26 of 208 positions per core), bf16
    matmul operands, one activation table set (exp/tanh only; silu and
    sigmoid are built from tanh, inverse sqrt is Quake-init Newton on the
    vector engine).
  - K-sharded final matmul: core c owns contraction rows for its 26
    positions, streams its [12, 96, 26, 768] bf16 weight shard from HBM
    with a deep prefetch ring that starts at t=0.
  - The partial xf [768, 96] is combined with 4 chunked ReduceScatters
    (s-major row blocks) overlapped with the weight-stream matmuls; each
    core receives only its 96 tail tokens.
  - Token-sharded TTT tail (96 tokens/core): fused projection matmul
    (z|kp|vp|q2 in one 481-wide rhs), gradient partials via ones-column
    tricks, one small grad AllReduce, replicated param update, per-core
    retrieve; the host gathers the 8 output slices.
"""

import math

import numpy as np
import ml_dtypes

import concourse.bass as bass
import concourse.mybir as mybir
import concourse.tile as tile
from concourse import bacc
from concourse import bass_utils
from concourse.bass import ds
from concourse.masks import make_identity

F32 = mybir.dt.float32
BF16 = mybir.dt.bfloat16
I32 = mybir.dt.int32
AF = mybir.ActivationFunctionType
OP = mybir.AluOpType

B, S, H, PM, FF, NH = 8, 96, 96, 16, 2048, 2
ALPHA, THETA = 0.999, 0.3
L = PM + 2 * S            # 208 encoder tokens per batch
NC = 8
LSH = L // NC             # 26 positions per core
DK = LSH * H              # 2496 contraction rows per core
DOUT = S * H              # 9216
TQ = B * S                # 768 tokens
HD = H // NH              # 48
NTOK = B * L              # 1664
TSH = B * LSH             # 208 sharded tokens per core
CH = TQ // 2              # 384
OC = 768                  # big-matmul output chunk (8 s-positions)
NOC = DOUT // OC          # 12
NG = 4                    # ReduceScatter groups (3 chunks each)
TPC = TQ // NC            # 96 tail tokens per core
H2 = 2 * H                # 192
WCAT = H2 + (H + 1) + H + H  # 481: z | kp+ones | vp' | q2

CFG = {"w_bufs": 36, "ll2": 2}

_CACHE = {}

MAGIC = 0x5F3759DF


def _mm(nc, out, lhsT, rhs, start, stop):
    nc.tensor.matmul(out, lhsT, rhs, start=start, stop=stop)


def _rsqrt(nc, work, pool_tag, s_ap, out_ap, iters=2):
    """out = 1/sqrt(s) via Quake init + Newton, all on the vector engine.

    s_ap: f32 AP [P, T] (strictly positive). out_ap: f32 AP same shape.
    """
    shp = list(s_ap.shape)
    y = work.tile(shp, F32, tag=f"{pool_tag}_y", name="rs_y", bufs=1)
    t = work.tile(shp, F32, tag=f"{pool_tag}_t", name="rs_t", bufs=1)
    s = work.tile(shp, F32, tag=f"{pool_tag}_s", name="rs_s", bufs=1)
    nc.vector.tensor_copy(s[:], s_ap)
    # y0 bits = MAGIC - (bits(s) >> 1)  ==  ((bits(s)>>1) ^ -1) + (MAGIC+1)
    nc.vector.tensor_scalar(y[:].bitcast(I32), s[:].bitcast(I32),
                            1, None, OP.logical_shift_right)
    nc.vector.tensor_scalar(y[:].bitcast(I32), y[:].bitcast(I32),
                            -1, None, OP.bitwise_xor)
    nc.vector.tensor_scalar(y[:].bitcast(I32), y[:].bitcast(I32),
                            MAGIC + 1, None, OP.add)
    for it in range(iters):
        nc.vector.tensor_mul(t[:], s[:], y[:])
        nc.vector.tensor_mul(t[:], t[:], y[:])
        nc.vector.tensor_scalar(t[:], t[:], -0.5, 1.5, OP.mult, OP.add)
        nc.vector.tensor_mul(out_ap if it == iters - 1 else y[:], y[:], t[:])


def build(cfg):
    nc = bacc.Bacc("TRN2", target_bir_lowering=False, debug=False, num_devices=NC)

    def din(name, shape, dt=F32):
        return nc.dram_tensor(name, shape, dt, kind="ExternalInput")

    dd = dict(
        xT_d=din("xT", [H, TQ]),
        xTb_d=din("xTb", [H, TQ], BF16),
        pmT_d=din("pmT", [H, PM]),
        qwTb_d=din("qwTb", [H, H], BF16),
        qb_d=din("qb", [H, 1]),
        ipqT_d=din("ipqT", [H, NH, HD], BF16),
        ipkT_d=din("ipkT", [H, NH, HD], BF16),
        ipvT_d=din("ipvT", [H, H], BF16),
        ipqb_d=din("ipqb", [HD, NH, 1]),
        ipkb_d=din("ipkb", [HD, NH, 1]),
        ipvb_d=din("ipvb", [1, H]),
        opT_d=din("opT", [HD, NH, H], BF16),
        opb_d=din("opb", [H, 1]),
        ln1w_d=din("ln1w", [H, 1]), ln1b_d=din("ln1b", [H, 1]),
        ln2w_d=din("ln2w", [H, 1]), ln2b_d=din("ln2b", [H, 1]),
        f1T_d=din("f1T", [H, FF], BF16),
        f1b_d=din("f1b", [128, FF // 128, 1]),
        f1bh_d=din("f1bh", [128, FF // 128, 1]),
        f2T_d=din("f2T", [128, FF // 128, H], BF16),
        f2b_d=din("f2b", [H, 1]),
        m1Tb_d=din("m1Tb", [H + 1, H2], BF16),
        m1T_d=din("m1T", [H + 1, H2]),
        m1b_d=din("m1b", [H, 2, 1]),
        m2Tb_d=din("m2Tb", [H + 1, 2, H], BF16),
        m2T_d=din("m2T", [H + 1, 2, H]),
        m2b_d=din("m2b", [H, 1]),
        m2wb_d=din("m2wb", [H, H2], BF16),
        wcat_d=din("wcat", [H + 1, WCAT], BF16),
        fbs_d=din("fbs", [TPC, H]),
        wt_d=din("WTc", [NOC, 128, 20, OC], BF16),
    )
    dd["out_d"] = nc.dram_tensor("outf", [H, TQ], F32, kind="ExternalOutput")
    if cfg.get("debug"):
        for nm, shp in [("d_xcf", [H, B, L]), ("d_x1n", [H, TSH]),
                        ("d_xe", [H, TSH]), 
                        ("d_grads", [H + 1, 4 * H]),
                        ("d_q2T", [H, TPC]), ("d_nm1T", [H, H2]),
                        ("d_xff", [H, TPC]), ("d_kf", [HD, NH, B, L]),
                        ("d_h", [TPC, 194]), ("d_dz", [TPC, H2])]:
            dd[nm] = nc.dram_tensor(nm, shp, F32, kind="ExternalOutput")
        for nm, shp in [("d_arin", [TQ, H]), ("d_rs", [TQ, H])]:
            dd[nm] = nc.dram_tensor(nm, shp, BF16, kind="ExternalOutput")

    with tile.TileContext(nc) as tc:
        _body(nc, tc, dd, cfg)
    nc.compile()
    return nc


def _body(nc, tc, dd, cfg):
    from contextlib import ExitStack
    stack = ExitStack()

    def pool(name, bufs, space="SBUF"):
        return stack.enter_context(tc.tile_pool(name=name, bufs=bufs, space=space))

    const = pool("const", 1)
    big = pool("big", 1)
    work = pool("work", 1)
    wstr = pool("wstr", cfg["w_bufs"])
    psxp = pool("psxp", 2, "PSUM")
    pss = pool("pss", 3, "PSUM")
    pscp = pool("pscp", 1, "PSUM")
    dram = pool("dram", 1, "DRAM")
    ar_in = dram.tile([TQ, H], BF16, tag="ar_in", name="ar_in")
    ar_out = dram.tile([TQ, H], BF16, tag="ar_out", name="ar_out")
    gr_in = dram.tile([H + 1, 4 * H], BF16, tag="gr_in", name="gr_in")
    gr_out = dram.tile([H + 1, 4 * H], BF16, tag="gr_out", name="gr_out",
                       addr_space="Shared")

    def ld(dram_t, tag):
        t = const.tile(list(dram_t.shape), dram_t.dtype, tag=tag, name=tag)
        nc.sync.dma_start(t[:], dram_t[:])
        return t

    qwTb = ld(dd["qwTb_d"], "qwTb"); qb = ld(dd["qb_d"], "qb")
    ipqT = ld(dd["ipqT_d"], "ipqT"); ipkT = ld(dd["ipkT_d"], "ipkT")
    ipvT = ld(dd["ipvT_d"], "ipvT")
    ipqb = ld(dd["ipqb_d"], "ipqb"); ipkb = ld(dd["ipkb_d"], "ipkb")
    opT = ld(dd["opT_d"], "opT"); opb = ld(dd["opb_d"], "opb")
    ln1w = ld(dd["ln1w_d"], "ln1w"); ln1b = ld(dd["ln1b_d"], "ln1b")
    ln2w = ld(dd["ln2w_d"], "ln2w"); ln2b = ld(dd["ln2b_d"], "ln2b")
    f1T = ld(dd["f1T_d"], "f1T"); f1b = ld(dd["f1b_d"], "f1b")
    f1bh = ld(dd["f1bh_d"], "f1bh")
    f2T = ld(dd["f2T_d"], "f2T"); f2b = ld(dd["f2b_d"], "f2b")
    m1Tb = ld(dd["m1Tb_d"], "m1Tb"); m1T = ld(dd["m1T_d"], "m1T")
    m1b = ld(dd["m1b_d"], "m1b")
    m2Tb = ld(dd["m2Tb_d"], "m2Tb"); m2T = ld(dd["m2T_d"], "m2T")
    m2b = ld(dd["m2b_d"], "m2b")
    m2wb = ld(dd["m2wb_d"], "m2wb")
    wcat = ld(dd["wcat_d"], "wcat")
    fbs = ld(dd["fbs_d"], "fbs")
    pmT = ld(dd["pmT_d"], "pmT")

    vb_bc = const.tile([128, H], F32, tag="vb_bc", name="vb_bc")
    nc.sync.dma_start(vb_bc[:], dd["ipvb_d"][:].to_broadcast([128, H]))

    ident = const.tile([128, 128], F32, tag="ident", name="ident")
    make_identity(nc, ident[:])
    identb = const.tile([128, 128], BF16, tag="identb", name="identb")
    nc.vector.tensor_copy(identb[:], ident[:])
    ones_col = const.tile([128, 1], F32, tag="ones_col", name="ones_col")
    nc.vector.memset(ones_col[:], 1.0)
    ones_row = const.tile([1, H], F32, tag="ones_row", name="ones_row")
    nc.vector.memset(ones_row[:], 1.0)
    zb = const.tile([128, 1], F32, tag="zb", name="zb")
    nc.vector.memset(zb[:], 0.0)
    eps1 = const.tile([1, 1], F32, tag="eps1", name="eps1")
    nc.vector.memset(eps1[:], 1e-5)

    pid = nc.partition_id()
    qoff = pid * LSH

    # ============ F1: front ============
    xTb = big.tile([H, TQ], BF16, tag="xTb", name="xTb")
    nc.sync.dma_start(xTb[:], dd["xTb_d"][:])

    xcf = big.tile([H, B, L], F32, tag="xcf", name="xcf")
    xcb = big.tile([H, B, L], BF16, tag="xcb", name="xcb")
    nc.vector.tensor_copy(xcf[:, :, 0:PM],
                          pmT[:].unsqueeze(1).to_broadcast([H, B, PM]))
    nc.gpsimd.tensor_copy(xcb[:, :, 0:PM],
                          pmT[:].unsqueeze(1).to_broadcast([H, B, PM]))
    nc.sync.dma_start(xcf[:, :, PM + S:L],
                        dd["xT_d"][:].rearrange("h (b s) -> h b s", b=B))
    nc.gpsimd.tensor_copy(xcb[:, :, PM + S:L],
                          xTb[:].rearrange("h (b s) -> h b s", b=B))

    # neural-memory retrieve for all 768 tokens -> nmm region of xcf
    for c in range(2):
        sl = slice(c * CH, (c + 1) * CH)
        ps = pss.tile([H, CH], F32, tag="ps", name="ps_q1")
        _mm(nc, ps[:], qwTb[:], xTb[:, sl], True, True)
        q1c = work.tile([H, CH], F32, tag="q1c", name="q1c", bufs=2)
        nc.scalar.activation(q1c[:], ps[:], AF.Identity, bias=qb[:])
        sq = work.tile([H, CH], F32, tag="l2sq", name="l2sq", bufs=2)
        nc.vector.tensor_mul(sq[:], q1c[:], q1c[:])
        ps_s = pss.tile([1, CH], F32, tag="ps", name="ps_l2s")
        _mm(nc, ps_s[:], ones_col[:H, :], sq[:], True, True)
        rsq = work.tile([1, CH], F32, tag="l2r", name="l2r", bufs=1)
        _rsqrt(nc, work, "l2", ps_s[:], rsq[:])
        ps_b = pss.tile([H, CH], F32, tag="ps", name="ps_l2b")
        _mm(nc, ps_b[:], ones_row[:], rsq[:], True, True)
        qn = work.tile([H, CH], F32, tag="qn", name="qn", bufs=2)
        nc.vector.tensor_mul(qn[:], q1c[:], ps_b[:])
        qry = work.tile([H, CH], BF16, tag="qry", name="qry", bufs=2)
        nc.scalar.activation(qry[:], qn[:], AF.Silu, bias=zb[:H, :])
        h1 = []
        for m in range(2):
            psm = pss.tile([H, CH], F32, tag="ps", name="ps_h1")
            _mm(nc, psm[:], m1Tb[0:H, m * H:(m + 1) * H], qry[:], True, True)
            h1c = work.tile([H, CH], BF16, tag="h1c", name="h1c", bufs=2)
            nc.scalar.activation(h1c[:], psm[:], AF.Silu, bias=m1b[:, m, :])
            h1.append(h1c)
        ps2 = pss.tile([H, CH], F32, tag="ps", name="ps_nmm")
        _mm(nc, ps2[:], m2Tb[0:H, 0, :], h1[0][:], True, False)
        _mm(nc, ps2[:], m2Tb[0:H, 1, :], h1[1][:], False, True)
        nc.scalar.activation(
            xcf[:, c * 4:(c + 1) * 4, PM:PM + S],
            ps2[:].rearrange("h (b s) -> h b s", b=4), AF.Identity, bias=m2b[:])
        nc.gpsimd.tensor_copy(xcb[:, c * 4:(c + 1) * 4, PM:PM + S],
                              xcf[:, c * 4:(c + 1) * 4, PM:PM + S])

    # k (all tokens), q (own 26 positions)
    kf = big.tile([HD, NH, B, L], BF16, tag="kf", name="kf")
    q_sel = big.tile([HD, NH, B, LSH], BF16, tag="q_sel", name="q_sel")
    xcb_flat = xcb[:].rearrange("h b l -> h (b l)")
    ECH = NTOK // 4
    for hh in range(NH):
        for c in range(4):
            sl = slice(c * ECH, (c + 1) * ECH)
            ps = pss.tile([HD, ECH], F32, tag="ps", name="ps_k")
            _mm(nc, ps[:], ipkT[:, hh, :], xcb_flat[:, sl], True, True)
            nc.scalar.activation(
                kf[:].rearrange("d n b l -> d n (b l)")[:, hh, sl],
                ps[:], AF.Identity, bias=ipkb[:, hh, :])
        psq = pss.tile([HD, TSH], F32, tag="ps", name="ps_q")
        _mm(nc, psq[:], ipqT[:, hh, :], xcb[:, :, ds(qoff, LSH)], True, True)
        nc.scalar.activation(q_sel[:, hh, :, :],
                             psq[:].rearrange("d (b l) -> d b l", b=B),
                             AF.Identity, bias=ipqb[:, hh, :])

    # v token-major per batch: [128, B, H] + [80, B, H] (bf16)
    v_tm0 = big.tile([128, B, H], BF16, tag="v_tm0", name="v_tm0")
    v_tm1 = big.tile([80, B, H], BF16, tag="v_tm1", name="v_tm1")
    for b in range(B):
        for tt, dst, npart in ((0, v_tm0, 128), (1, v_tm1, 80)):
            ps = pss.tile([128, H], F32, tag="ps", name="ps_v")
            toks = slice(tt * 128, tt * 128 + npart)
            _mm(nc, ps[:npart, :], xcb[:, b, toks], ipvT[:], True, True)
            nc.vector.tensor_add(dst[:, b, :], ps[:npart, :], vb_bc[:npart, :])

    # attention: scores/exp batched over pairs of batches per head
    of = big.tile([HD, NH, B, LSH], BF16, tag="of", name="of")
    for hh in range(NH):
        for bg in range(4):
            sc2 = pss.tile([64, L], F32, tag="ps", name="sc2")
            for bq in range(2):
                b = bg * 2 + bq
                _mm(nc, sc2[bq * 32:bq * 32 + LSH, :],
                    q_sel[:, hh, b, :], kf[:, hh, b, :], True, True)
            e2a = work.tile([64, L], BF16, tag="e2a", name="e2a", bufs=2)
            den2 = work.tile([64, 1], F32, tag="den2", name="den2", bufs=2)
            nc.scalar.activation(e2a[:], sc2[:], AF.Exp, bias=zb[:64, :],
                                 accum_out=den2[:])
            rden = work.tile([64, 1], F32, tag="rden", name="rden", bufs=2)
            nc.vector.reciprocal(rden[:], den2[:])
            a2 = work.tile([64, L], BF16, tag="a2", name="a2", bufs=2)
            nc.vector.tensor_scalar_mul(a2[:], e2a[:], rden[:])
            at2 = work.tile([128, 2, 64], BF16, tag="at2", name="at2", bufs=2)
            for kc, npart in ((0, 128), (1, 80)):
                pst = pss.tile([128, 64], BF16, tag="ps", name="ps_t")
                nc.tensor.transpose(pst[:npart, :],
                                    a2[:, kc * 128:kc * 128 + npart],
                                    identb[:64, :64])
                nc.vector.tensor_copy(at2[:npart, kc, :], pst[:npart, :])
            for bq in range(2):
                b = bg * 2 + bq
                ps_o = pss.tile([HD, LSH], F32, tag="ps", name="ps_o")
                for tt, vsrc, npart in ((0, v_tm0, 128), (1, v_tm1, 80)):
                    _mm(nc, ps_o[:], vsrc[:, b, hh * HD:(hh + 1) * HD],
                        at2[:npart, tt, bq * 32:bq * 32 + LSH], tt == 0, tt == 1)
                nc.scalar.copy(of[:, hh, b, :], ps_o[:])

    # out_proj + residual
    ps_op = pss.tile([H, TSH], F32, tag="ps", name="ps_op")
    for hh in range(NH):
        _mm(nc, ps_op[:], opT[:, hh, :],
            of[:, hh, :, :].rearrange("d b l -> d (b l)"), hh == 0, hh == 1)
    x1 = big.tile([H, TSH], F32, tag="x1", name="x1")
    nc.vector.tensor_scalar_add(x1[:], ps_op[:], opb[:])
    nc.vector.tensor_add(x1[:].rearrange("h (b l) -> h b l", b=B),
                         x1[:].rearrange("h (b l) -> h b l", b=B),
                         xcf[:, :, ds(qoff, LSH)])

    x1n = big.tile([H, TSH], F32, tag="x1n", name="x1n")
    _layernorm(nc, pss, work, x1[:], x1n[:], ln1w, ln1b, ones_col, ones_row, eps1)
    x1nb = big.tile([H, TSH], BF16, tag="x1nb", name="x1nb")
    nc.gpsimd.tensor_copy(x1nb[:], x1n[:])

    ps2f = pss.tile([H, TSH], F32, tag="ps", name="ps_ff2")
    for m in range(FF // 128):
        psf = pss.tile([128, TSH], F32, tag="ps", name="ps_ff1")
        _mm(nc, psf[:], f1T[:, m * 128:(m + 1) * 128], x1nb[:], True, True)
        h_ffn = work.tile([128, TSH], BF16, tag="h_ffn", name="h_ffn", bufs=3)
        nc.scalar.activation(h_ffn[:], psf[:], AF.Silu, bias=f1b[:, m, :])
        _mm(nc, ps2f[:], f2T[:, m, :], h_ffn[:], m == 0, m == FF // 128 - 1)
    x2 = big.tile([H, TSH], F32, tag="x2", name="x2")
    nc.vector.tensor_scalar_add(x2[:], ps2f[:], f2b[:])
    nc.vector.tensor_add(x2[:], x2[:], x1n[:])

    e2 = big.tile([H, TSH], F32, tag="e2", name="e2")
    _layernorm(nc, pss, work, x2[:], e2[:], ln2w, ln2b, ones_col, ones_row, eps1)
    # repack xe to K=128 lhsT tiles [128, 20, 8] via a DRAM round-trip:
    # per-batch transpose+silu -> [26, b, 96] -> DRAM rows [8, 2560]
    # (64-el zero pad per batch) -> one strided gather.
    xe_dram = dram.tile([B, 2560], F32, tag="xe_dram", name="xe_dram")
    xe_tokB = big.tile([LSH, B, H], F32, tag="xe_tokB", name="xe_tokB")
    e23 = e2[:].rearrange("h (b l) -> h b l", b=B)
    for b in range(B):
        pst = pss.tile([LSH, H], F32, tag="ps", name="ps_xet")
        nc.tensor.transpose(pst[:], e23[:, b, :], ident[:H, :H])
        nc.scalar.activation(xe_tokB[:, b, :], pst[:], AF.Silu, bias=zb[:LSH, :])
    zpad = const.tile([B, 64], F32, tag="zpad", name="zpad")
    nc.vector.memset(zpad[:], 0.0)
    nc.scalar.dma_start(xe_dram[:, 2496:2560], zpad[:])
    nc.scalar.dma_start(
        xe_dram[:, 0:DK].rearrange("b (l h) -> l b h", h=H), xe_tokB[:])
    xe128f = big.tile([128, B, 20], F32, tag="xe128f", name="xe128f")
    nc.scalar.dma_start(
        xe128f[:], xe_dram[:, :].rearrange("b (c p) -> p b c", p=128))
    xe128 = big.tile([128, B, 20], BF16, tag="xe128", name="xe128")
    nc.vector.tensor_copy(xe128[:], xe128f[:])

    if cfg.get("debug"):
        nc.sync.dma_start(dd["d_xcf"][:], xcf[:])
        nc.sync.dma_start(dd["d_x1n"][:], x1n[:])
        dxe = work.tile([H, TSH], F32, tag="dxe", name="dxe")
        nc.vector.tensor_copy(dxe[:], xeb[:])
        nc.sync.dma_start(dd["d_xe"][:], dxe[:])
        dkf = work.tile([HD, NH, B, L], F32, tag="dkf", name="dkf")
        nc.vector.tensor_copy(dkf[:], kf[:])
        nc.sync.dma_start(dd["d_kf"][:], dkf[:])

    # ============ F2: big matmul (K-sharded) + chunked ReduceScatter ============
    # ar_in rows are (s, b)-major; the chunk DMA writes through a
    # batch-first view so src (SBUF partition=b) and dst dims align.
    ar_in_b = ar_in[:].rearrange("(s b) h -> b s h", b=B)   # [8, 96, 96] strided
    wt4 = dd["wt_d"][:]
    for ci in range(NOC):
        psx = psxp.tile([B, OC], F32, tag="psx", name="psx")
        for k0 in range(0, 20, 2):
            wt = wstr.tile([128, 2, OC], BF16, tag="wt", name="wt")
            nc.sync.dma_start(wt[:], wt4[ci, :, k0:k0 + 2, :])
            for k1 in range(2):
                kg = k0 + k1
                for j0 in range(0, OC, 512):
                    j1 = min(j0 + 512, OC)
                    _mm(nc, psx[:, j0:j1], xe128[:, :, kg], wt[:, k1, j0:j1],
                        kg == 0, kg == 19)
        xfp = work.tile([B, OC], BF16, tag="xfp", name="xfp", bufs=2)
        nc.scalar.copy(xfp[:], psx[:])
        nc.scalar.dma_start(ar_in_b[:, 8 * ci:8 * ci + 8, :],
                            xfp[:].rearrange("b (s h) -> b s h", s=8))
    nc.gpsimd.collective_compute(
        "AllReduce", OP.add,
        replica_groups=[list(range(NC))],
        ins=[ar_in[:, :].opt()],
        outs=[ar_out[:, :].opt()],
    )

    # ============ F3: tail (96 tokens per core) ============
    xf_bf = big.tile([TPC, H], BF16, tag="xf_bf", name="xf_bf")
    for g in range(NG):
        nc.scalar.dma_start(xf_bf[24 * g:24 * g + 24, :],
                            ar_out[ds(192 * g + pid * 24, 24), :])
    xf_tm = big.tile([TPC, H], F32, tag="xf_tm", name="xf_tm")
    nc.vector.tensor_add(xf_tm[:], xf_bf[:], fbs[:])

    ps_xt = pss.tile([H, TPC], F32, tag="ps", name="ps_xt")
    nc.tensor.transpose(ps_xt[:], xf_tm[:], ident[:TPC, :TPC])
    xffT = big.tile([H, TPC], F32, tag="xffT", name="xffT")
    nc.vector.tensor_copy(xffT[:], ps_xt[:])
    xffTb = big.tile([H + 1, TPC], BF16, tag="xffTb", name="xffTb")
    nc.vector.tensor_copy(xffTb[0:H, :], ps_xt[:])
    nc.vector.memset(xffTb[H:H + 1, :], 1.0)

    psc = pscp.tile([TPC, WCAT], F32, tag="psc", name="psc")
    _mm(nc, psc[:], xffTb[:], wcat[:], True, True)
    zsl = psc[:, 0:H2]
    kpsl = psc[:, H2:H2 + H + 1]
    vpsl = psc[:, H2 + H + 1:H2 + 2 * H + 1]
    q2sl = psc[:, H2 + 2 * H + 1:WCAT]

    kp_b = big.tile([TPC, H + 1], BF16, tag="kp_b", name="kp_b")
    nc.vector.tensor_copy(kp_b[:], kpsl)
    q2r = big.tile([TPC, H], F32, tag="q2r", name="q2r")
    nc.vector.tensor_copy(q2r[:], q2sl)
    vp_sb = big.tile([TPC, H], F32, tag="vp_sb", name="vp_sb")
    nc.vector.tensor_copy(vp_sb[:], vpsl)

    # h = silu(z), sp = sig(z)*(1 + z - h)   (z includes fused bias)
    thz = work.tile([TPC, H2], F32, tag="thz", name="thz")
    nc.scalar.activation(thz[:], zsl, AF.Tanh, bias=zb[:TPC, :], scale=0.5)
    nc.vector.tensor_scalar(thz[:], thz[:], 0.5, 0.5, OP.mult, OP.add)
    h_tm = big.tile([TPC, 194], BF16, tag="h_tm", name="h_tm")
    nc.vector.tensor_mul(h_tm[:, 0:H], psc[:, 0:H], thz[:, 0:H])
    nc.vector.tensor_mul(h_tm[:, H + 1:2 * H + 1], psc[:, H:H2], thz[:, H:H2])
    nc.vector.memset(h_tm[:, H:H + 1], 1.0)
    nc.vector.memset(h_tm[:, 2 * H + 1:2 * H + 2], 1.0)
    sp = big.tile([TPC, H2], F32, tag="sp", name="sp")
    nc.vector.tensor_scalar(sp[:], thz[:], -1.0, 1.0, OP.mult, OP.add)
    nc.vector.tensor_mul(sp[:], sp[:], zsl)
    nc.vector.tensor_scalar_add(sp[:], sp[:], 1.0)
    nc.vector.tensor_mul(sp[:], sp[:], thz[:])

    # pred via h_fm
    h_fm = big.tile([H, 2, TPC], BF16, tag="h_fm", name="h_fm")
    for m in range(2):
        pst = pss.tile([H, TPC], BF16, tag="ps", name="ps_hf")
        nc.tensor.transpose(pst[:], h_tm[:, m * (H + 1):m * (H + 1) + H],
                            identb[:TPC, :TPC])
        nc.vector.tensor_copy(h_fm[:, m, :], pst[:])
    psp = pss.tile([TPC, H], F32, tag="ps", name="ps_pred")
    _mm(nc, psp[:], h_fm[:, 0, :], m2Tb[0:H, 0, :], True, False)
    _mm(nc, psp[:], h_fm[:, 1, :], m2Tb[0:H, 1, :], False, True)
    dpr_b = big.tile([TPC, H], BF16, tag="dpr_b", name="dpr_b")
    nc.vector.tensor_sub(dpr_b[:], psp[:], vp_sb[:])

    pst2 = pss.tile([H, TPC], BF16, tag="ps", name="ps_dprT")
    nc.tensor.transpose(pst2[:], dpr_b[:], identb[:TPC, :TPC])
    dprT = big.tile([H, TPC], BF16, tag="dprT", name="dprT")
    nc.vector.tensor_copy(dprT[:], pst2[:])

    psd = pss.tile([TPC, H2], F32, tag="ps", name="ps_dz")
    _mm(nc, psd[:], dprT[:], m2wb[:], True, True)
    dz_b = big.tile([TPC, H2], BF16, tag="dz_b", name="dz_b")
    nc.vector.tensor_mul(dz_b[:], psd[:], sp[:])

    # grads: g1 [97, 192] = kp_aug^T dz ; g2 [97, 2, 96] = h_aug^T dpr
    psg1 = pss.tile([H + 1, H2], F32, tag="ps", name="ps_g1")
    _mm(nc, psg1[:], kp_b[:], dz_b[:], True, True)
    psg2 = pss.tile([H + 1, 2, H], F32, tag="ps", name="ps_g2")
    for m in range(2):
        _mm(nc, psg2[:, m, :], h_tm[:, m * (H + 1):(m + 1) * (H + 1)],
            dpr_b[:], True, True)
    grads = big.tile([H + 1, 4 * H], BF16, tag="grads", name="grads")
    nc.vector.tensor_copy(grads[:, 0:H2], psg1[:])
    nc.vector.tensor_copy(grads[:, H2:4 * H],
                          psg2[:].rearrange("p m h -> p (m h)"))
    nc.scalar.dma_start(gr_in[:, :], grads[:])
    nc.gpsimd.collective_compute(
        "AllReduce", OP.add,
        replica_groups=[list(range(NC))],
        ins=[gr_in[:, :].opt()],
        outs=[gr_out[:, :].opt()],
    )

    # q2 = l2norm over features (token-major) then transpose
    sqq = work.tile([TPC, H], F32, tag="sqq", name="sqq")
    nc.vector.tensor_mul(sqq[:], q2r[:], q2r[:])
    ssum = work.tile([TPC, 1], F32, tag="ssum", name="ssum")
    nc.vector.reduce_sum(ssum[:], sqq[:], axis=mybir.AxisListType.X)
    rs2 = work.tile([TPC, 1], F32, tag="rs2", name="rs2")
    _rsqrt(nc, work, "l2t", ssum[:], rs2[:])
    q2n = work.tile([TPC, H], BF16, tag="q2n", name="q2n")
    nc.vector.tensor_scalar_mul(q2n[:], q2r[:], rs2[:])
    ps_qt = pss.tile([H, TPC], BF16, tag="ps", name="ps_q2T")
    nc.tensor.transpose(ps_qt[:], q2n[:], identb[:TPC, :TPC])
    q2T = big.tile([H + 1, TPC], BF16, tag="q2T", name="q2T")
    nc.vector.tensor_copy(q2T[0:H, :], ps_qt[:])
    nc.vector.memset(q2T[H:H + 1, :], 1.0)

    # ============ F4: finalize ============
    grd = big.tile([H + 1, 4 * H], BF16, tag="grd", name="grd")
    nc.scalar.dma_start(grd[:], gr_out[:, :])
    THP = THETA * 2.0 / (TQ * H)

    if cfg.get("debug"):
        nc.sync.dma_start(dd["d_arin"][:], ar_in[:, :])
        nc.sync.dma_start(dd["d_rs"][:], ar_out[:, :])
        nc.sync.dma_start(dd["d_grads"][:], gr_out[:, :])
        nc.sync.dma_start(dd["d_xff"][:], xffT[:])
        dq2 = work.tile([H, TPC], F32, tag="dq2", name="dq2")
        nc.vector.tensor_copy(dq2[:], q2T[:])
        nc.sync.dma_start(dd["d_q2T"][:], dq2[:])
        dh = work.tile([TPC, 194], F32, tag="dh", name="dh")
        nc.vector.tensor_copy(dh[:], h_tm[:])
        nc.sync.dma_start(dd["d_h"][:], dh[:])
        ddz = work.tile([TPC, H2], F32, tag="ddz", name="ddz")
        nc.vector.tensor_copy(ddz[:], dz_b[:])
        nc.sync.dma_start(dd["d_dz"][:], ddz[:])

    nm1T = big.tile([H + 1, H2], F32, tag="nm1T", name="nm1T")
    tg1 = work.tile([H + 1, H2], F32, tag="tg1", name="tg1")
    nc.vector.tensor_scalar_mul(tg1[:], grd[:, 0:H2], THP)
    nc.vector.tensor_scalar_mul(nm1T[:], m1T[:], ALPHA)
    nc.vector.tensor_sub(nm1T[:], nm1T[:], tg1[:])
    nm1Tb = big.tile([H + 1, H2], BF16, tag="nm1Tb", name="nm1Tb")
    nc.vector.tensor_copy(nm1Tb[:], nm1T[:])
    if cfg.get("debug"):
        nc.sync.dma_start(dd["d_nm1T"][:], nm1T[0:H, :])

    nm2T = big.tile([H + 1, 2, H], BF16, tag="nm2T", name="nm2T")
    tg2 = work.tile([H + 1, 2, H], F32, tag="tg2", name="tg2")
    nc.vector.tensor_scalar_mul(tg2[:].rearrange("p m h -> p (m h)"),
                                grd[:, H2:4 * H], THP)
    tg2b = work.tile([H + 1, 2, H], F32, tag="tg2b", name="tg2b")
    nc.vector.tensor_scalar_mul(tg2b[:].rearrange("p m h -> p (m h)"),
                                m2T[:].rearrange("p m h -> p (m h)"), ALPHA)
    nc.vector.tensor_sub(nm2T[:].rearrange("p m h -> p (m h)"),
                         tg2b[:].rearrange("p m h -> p (m h)"),
                         tg2[:].rearrange("p m h -> p (m h)"))
    # chunk-1 bias row must not double-count nm2b
    nc.vector.memset(nm2T[H:H + 1, 1, :], 0.0)

    # retrieve with updated memory for own 96 tokens (biases ride the
    # augmented ones rows)
    uu = []
    for m in range(2):
        psu = pss.tile([H, TPC], F32, tag="ps", name="ps_u")
        _mm(nc, psu[:], nm1Tb[:, m * H:(m + 1) * H], q2T[:], True, True)
        thu = work.tile([H, TPC], F32, tag="thu", name="thu", bufs=2)
        nc.scalar.activation(thu[:], psu[:], AF.Tanh, bias=zb[:H, :], scale=0.5)
        nc.vector.tensor_scalar(thu[:], thu[:], 0.5, 0.5, OP.mult, OP.add)
        u_b = work.tile([H + 1, TPC], BF16, tag="u_b", name="u_b", bufs=2)
        nc.vector.tensor_mul(u_b[0:H, :], psu[:], thu[:])
        nc.vector.memset(u_b[H:H + 1, :], 1.0)
        uu.append(u_b)
    psy = pss.tile([H, TPC], F32, tag="ps", name="ps_y")
    _mm(nc, psy[:], nm2T[:, 0, :], uu[0][:], True, False)
    _mm(nc, psy[:], nm2T[:, 1, :], uu[1][:], False, True)
    thy = work.tile([H, TPC], F32, tag="thy", name="thy")
    nc.scalar.activation(thy[:], psy[:], AF.Tanh, bias=zb[:H, :], scale=0.5)
    nc.vector.tensor_scalar(thy[:], thy[:], 0.5, 0.5, OP.mult, OP.add)
    ot = work.tile([H, TPC], F32, tag="ot", name="ot")
    nc.vector.tensor_mul(ot[:], xffT[:], thy[:])

    out3 = dd["out_d"][:].rearrange("h (b s) -> h b s", b=B)
    ot_b = work.tile([H, B, NG * 3], F32, tag="ot_b", name="ot_b")
    nc.vector.tensor_copy(ot_b[:],
                          ot[:].rearrange("h (g sp b) -> h b (g sp)", g=NG, sp=3))
    for g in range(NG):
        nc.scalar.dma_start(
            out3[:, :, ds(24 * g + pid * 3, 3)],
            ot_b[:, :, 3 * g:3 * g + 3])

    stack.close()


def _layernorm(nc, pss, work, src_ap, dst_ap, w_ap, b_ap, ones_col, ones_row, eps1):
    """dst = LN(src) * w + b over the feature (partition) axis; [96, T] APs."""
    T = src_ap.shape[-1]
    ps_s = pss.tile([1, T], F32, tag="ps", name="ps_lns")
    _mm(nc, ps_s[:], ones_col[:H, :], src_ap, True, True)
    mean = work.tile([1, T], F32, tag="ln_mean", name="ln_mean")
    nc.vector.tensor_scalar_mul(mean[:], ps_s[:], 1.0 / H)
    sq = work.tile([H, T], F32, tag="ln_sq", name="ln_sq")
    nc.vector.tensor_mul(sq[:], src_ap, src_ap)
    ps_q = pss.tile([1, T], F32, tag="ps", name="ps_lnq")
    _mm(nc, ps_q[:], ones_col[:H, :], sq[:], True, True)
    var = work.tile([1, T], F32, tag="ln_var", name="ln_var")
    nc.scalar.activation(var[:], ps_q[:], AF.Identity, bias=eps1[:], scale=1.0 / H)
    m2t = work.tile([1, T], F32, tag="ln_m2", name="ln_m2")
    nc.vector.tensor_mul(m2t[:], mean[:], mean[:])
    nc.vector.tensor_sub(var[:], var[:], m2t[:])
    rstd = work.tile([1, T], F32, tag="ln_rstd", name="ln_rstd")
    _rsqrt(nc, work, "ln", var[:], rstd[:])
    nmr = work.tile([1, T], F32, tag="ln_nmr", name="ln_nmr")
    nc.vector.tensor_mul(nmr[:], mean[:], rstd[:])
    nc.vector.tensor_scalar_mul(nmr[:], nmr[:], -1.0)
    ps_a = pss.tile([H, T], F32, tag="ps", name="ps_lna")
    _mm(nc, ps_a[:], ones_row[:], rstd[:], True, True)
    ps_c = pss.tile([H, T], F32, tag="ps", name="ps_lnc")
    _mm(nc, ps_c[:], ones_row[:], nmr[:], True, True)
    t1 = work.tile([H, T], F32, tag="ln_t1", name="ln_t1")
    nc.vector.tensor_mul(t1[:], src_ap, ps_a[:])
    nc.vector.tensor_add(t1[:], t1[:], ps_c[:])
    nc.vector.tensor_scalar(dst_ap, t1[:], w_ap[:], b_ap[:], OP.mult, OP.add)


def prep_inmaps(inputs, cfg=None):
    cfg = cfg or CFG
    f32 = np.float32
    bf16 = ml_dtypes.bfloat16

    def T(a):
        return np.ascontiguousarray(np.asarray(a, f32).T)

    x = np.asarray(inputs["x"], f32)
    ipw = np.asarray(inputs["in_proj_w"], f32)   # [288, 96]
    ipb = np.asarray(inputs["in_proj_b"], f32)   # [288]
    sc = 1.0 / math.sqrt(HD)
    qw_part = ipw[0:H] * sc
    qb_part = ipb[0:H] * sc
    kw_part = ipw[H:2 * H]
    kb_part = ipb[H:2 * H]
    vw_part = ipw[2 * H:3 * H]
    vb_part = ipb[2 * H:3 * H]

    ipqT = np.ascontiguousarray(qw_part.T.reshape(H, NH, HD))
    ipkT = np.ascontiguousarray(kw_part.T.reshape(H, NH, HD))
    ipqb = np.ascontiguousarray(qb_part.reshape(NH, HD).T.reshape(HD, NH, 1))
    ipkb = np.ascontiguousarray(kb_part.reshape(NH, HD).T.reshape(HD, NH, 1))

    opw = np.asarray(inputs["out_proj_w"], f32)
    opT = np.ascontiguousarray(opw.T.reshape(NH, HD, H).transpose(1, 0, 2))

    f1b = np.asarray(inputs["ff1_b"], f32).reshape(FF // 128, 128, 1)
    f1b = np.ascontiguousarray(f1b.transpose(1, 0, 2))
    f2T = T(inputs["ff2_w"])
    f2T = np.ascontiguousarray(f2T.reshape(FF // 128, 128, H).transpose(1, 0, 2))

    m1w = np.asarray(inputs["m1_w"], f32)        # [192, 96]
    m1bv = np.asarray(inputs["m1_b"], f32)       # [192]
    m2w = np.asarray(inputs["m2_w"], f32)        # [96, 192]
    m2bv = np.asarray(inputs["m2_b"], f32)       # [96]
    kw = np.asarray(inputs["k_w"], f32)
    kb = np.asarray(inputs["k_b"], f32)
    vw = np.asarray(inputs["v_w"], f32)
    vb = np.asarray(inputs["v_b"], f32)
    qw = np.asarray(inputs["q_w"], f32)
    qbv = np.asarray(inputs["q_b"], f32)

    m1b = np.ascontiguousarray(m1bv.reshape(2, H, 1).transpose(1, 0, 2))
    m1T_aug = np.concatenate([T(m1w), m1bv.reshape(1, H2)], 0)        # [97, 192]
    m2T3 = np.ascontiguousarray(T(m2w).reshape(2, H, H).transpose(1, 0, 2))
    m2T_aug = np.zeros((H + 1, 2, H), f32)
    m2T_aug[0:H] = m2T3
    m2T_aug[H, 0] = m2bv                                              # chunk-0 bias row

    # Wcat [97, 481]: z | kp(+ones col) | vp' | q2
    wcat = np.zeros((H + 1, WCAT), f32)
    m1kw = m1w @ kw                               # [192, 96]
    wcat[0:H, 0:H2] = m1kw.T
    wcat[H, 0:H2] = m1w @ kb + m1bv
    wcat[0:H, H2:H2 + H] = kw.T
    wcat[H, H2:H2 + H] = kb
    wcat[H, H2 + H] = 1.0                         # ones column for g1 bias row
    wcat[0:H, H2 + H + 1:H2 + 2 * H + 1] = vw.T
    wcat[H, H2 + H + 1:H2 + 2 * H + 1] = vb - m2bv
    wcat[0:H, H2 + 2 * H + 1:WCAT] = qw.T
    wcat[H, H2 + 2 * H + 1:WCAT] = qbv

    fwT = np.ascontiguousarray(np.asarray(inputs["final_w"], f32).T)
    fb = np.asarray(inputs["final_b"], f32).reshape(S, H)

    col = lambda k: np.ascontiguousarray(np.asarray(inputs[k], f32).reshape(-1, 1))
    xTf = T(x.reshape(TQ, H))
    base = dict(
        xT=xTf, xTb=xTf.astype(bf16),
        pmT=T(inputs["persistent_memory"]),
        qwTb=T(qw).astype(bf16), qb=col("q_b"),
        ipqT=ipqT.astype(bf16), ipkT=ipkT.astype(bf16),
        ipvT=np.ascontiguousarray(vw_part.T).astype(bf16),
        ipqb=ipqb, ipkb=ipkb,
        ipvb=np.ascontiguousarray(vb_part.reshape(1, H)),
        opT=opT.astype(bf16), opb=col("out_proj_b"),
        ln1w=col("ln1_w"), ln1b=col("ln1_b"),
        ln2w=col("ln2_w"), ln2b=col("ln2_b"),
        f1T=T(inputs["ff1_w"]).astype(bf16), f1b=f1b,
        f1bh=np.ascontiguousarray(f1b * 0.5),
        f2T=f2T.astype(bf16), f2b=col("ff2_b"),
        m1Tb=m1T_aug.astype(bf16), m1T=m1T_aug,
        m1b=m1b,
        m2Tb=m2T_aug.astype(bf16), m2T=m2T_aug,
        m2b=col("m2_b"),
        m2wb=np.ascontiguousarray(m2w).astype(bf16),
        wcat=wcat.astype(bf16),
    )
    in_maps = []
    for c in range(NC):
        m = dict(base)
        shard = fwT[c * DK:(c + 1) * DK]                     # [(l h), 9216]
        shard_pad = np.concatenate([shard, np.zeros((64, DOUT), f32)], 0)
        packed = shard_pad.reshape(20, 128, NOC, OC).transpose(2, 1, 0, 3)
        m["WTc"] = np.ascontiguousarray(packed.astype(bf16))
        # fbs: final_b rows for this core's tokens in (g, sp, b) order
        ss = np.array([24 * g + 3 * c + d for g in range(NG) for d in range(3)])
        m["fbs"] = np.ascontiguousarray(
            np.repeat(fb[ss], B, axis=0))                    # [96, 96]
        in_maps.append(m)
    return in_maps


def get_nc(cfg=None):
    cfg = cfg or CFG
    key = tuple(sorted((k, str(v)) for k, v in cfg.items()))
    if key not in _CACHE:
        _CACHE[key] = build(cfg)
    return _CACHE[key]


def assemble(results):
    """Gather per-core output column slices into the full [B, S, H] output."""
    full = np.zeros((H, TQ), np.float32)
    for c in range(NC):
        outc = results[c]["outf"]                            # [96, 768]
        cols = np.array([b * S + 24 * g + 3 * c + d
                         for g in range(NG) for d in range(3) for b in range(B)])
        full[:, cols] = outc[:, cols]
    return np.ascontiguousarray(full.T).reshape(B, S, H)


def kernel(**inputs):
    nc = get_nc()
    in_maps = prep_inmaps(inputs)
    res = bass_utils.run_bass_kernel_spmd(
        nc, in_maps, core_ids=list(range(NC)), trace=False
    )
    return assemble(res.results)


if __name__ == "__main__":
    print("building...")
    get_nc()
    print("built")


# revision 27
# speedup vs baseline: 1.1073x; 1.0697x over previous
"""Trainium2 Bass kernel for nn_MACTitanLayer (MAC Titan layer, 8 cores).

Structure (v2):
  - Position-sharded encoder front (26 of 208 positions per core), bf16
    matmul operands, one activation table set (exp/tanh only; silu and
    sigmoid are built from tanh, inverse sqrt is Quake-init Newton on the
    vector engine).
  - K-sharded final matmul: core c owns contraction rows for its 26
    positions, streams its [12, 96, 26, 768] bf16 weight shard from HBM
    with a deep prefetch ring that starts at t=0.
  - The partial xf [768, 96] is combined with 4 chunked ReduceScatters
    (s-major row blocks) overlapped with the weight-stream matmuls; each
    core receives only its 96 tail tokens.
  - Token-sharded TTT tail (96 tokens/core): fused projection matmul
    (z|kp|vp|q2 in one 481-wide rhs), gradient partials via ones-column
    tricks, one small grad AllReduce, replicated param update, per-core
    retrieve; the host gathers the 8 output slices.
"""

import math

import numpy as np
import ml_dtypes

import concourse.bass as bass
import concourse.mybir as mybir
import concourse.tile as tile
from concourse import bacc
from concourse import bass_utils
from concourse.bass import ds
from concourse.masks import make_identity

F32 = mybir.dt.float32
BF16 = mybir.dt.bfloat16
I32 = mybir.dt.int32
AF = mybir.ActivationFunctionType
OP = mybir.AluOpType

B, S, H, PM, FF, NH = 8, 96, 96, 16, 2048, 2
ALPHA, THETA = 0.999, 0.3
L = PM + 2 * S            # 208 encoder tokens per batch
NC = 8
LSH = L // NC             # 26 positions per core
DK = LSH * H              # 2496 contraction rows per core
DOUT = S * H              # 9216
TQ = B * S                # 768 tokens
HD = H // NH              # 48
NTOK = B * L              # 1664
TSH = B * LSH             # 208 sharded tokens per core
CH = TQ // 2              # 384
OC = 768                  # big-matmul output chunk (8 s-positions)
NOC = DOUT // OC          # 12
NG = 4                    # ReduceScatter groups (3 chunks each)
TPC = TQ // NC            # 96 tail tokens per core
H2 = 2 * H                # 192
WCAT = H2 + (H + 1) + H + H  # 481: z | kp+ones | vp' | q2

CFG = {"w_bufs": 36, "ll2": 2}

_CACHE = {}

MAGIC = 0x5F3759DF


def _mm(nc, out, lhsT, rhs, start, stop):
    nc.tensor.matmul(out, lhsT, rhs, start=start, stop=stop)


def _rsqrt(nc, work, pool_tag, s_ap, out_ap, iters=2):
    """out = 1/sqrt(s) via Quake init + Newton, all on the vector engine.

    s_ap: f32 AP [P, T] (strictly positive). out_ap: f32 AP same shape.
    """
    shp = list(s_ap.shape)
    y = work.tile(shp, F32, tag=f"{pool_tag}_y", name="rs_y", bufs=1)
    t = work.tile(shp, F32, tag=f"{pool_tag}_t", name="rs_t", bufs=1)
    s = work.tile(shp, F32, tag=f"{pool_tag}_s", name="rs_s", bufs=1)
    nc.vector.tensor_copy(s[:], s_ap)
    # y0 bits = MAGIC - (bits(s) >> 1)  ==  ((bits(s)>>1) ^ -1) + (MAGIC+1)
    nc.vector.tensor_scalar(y[:].bitcast(I32), s[:].bitcast(I32),
                            1, None, OP.logical_shift_right)
    nc.vector.tensor_scalar(y[:].bitcast(I32), y[:].bitcast(I32),
                            -1, None, OP.bitwise_xor)
    nc.vector.tensor_scalar(y[:].bitcast(I32), y[:].bitcast(I32),
                            MAGIC + 1, None, OP.add)
    for it in range(iters):
        nc.vector.tensor_mul(t[:], s[:], y[:])
        nc.vector.tensor_mul(t[:], t[:], y[:])
        nc.vector.tensor_scalar(t[:], t[:], -0.5, 1.5, OP.mult, OP.add)
        nc.vector.tensor_mul(out_ap if it == iters - 1 else y[:], y[:], t[:])


def build(cfg):
    nc = bacc.Bacc("TRN2", target_bir_lowering=False, debug=False, num_devices=NC)

    def din(name, shape, dt=F32):
        return nc.dram_tensor(name, shape, dt, kind="ExternalInput")

    dd = dict(
        xT_d=din("xT", [H, TQ]),
        xTb_d=din("xTb", [H, TQ], BF16),
        pmT_d=din("pmT", [H, PM]),
        qwTb_d=din("qwTb", [H, H], BF16),
        qb_d=din("qb", [H, 1]),
        ipqT_d=din("ipqT", [H, NH, HD], BF16),
        ipkT_d=din("ipkT", [H, NH, HD], BF16),
        ipvT_d=din("ipvT", [H, H], BF16),
        ipqb_d=din("ipqb", [HD, NH, 1]),
        ipkb_d=din("ipkb", [HD, NH, 1]),
        ipvb_d=din("ipvb", [1, H]),
        opT_d=din("opT", [HD, NH, H], BF16),
        opb_d=din("opb", [H, 1]),
        ln1w_d=din("ln1w", [H, 1]), ln1b_d=din("ln1b", [H, 1]),
        ln2w_d=din("ln2w", [H, 1]), ln2b_d=din("ln2b", [H, 1]),
        f1T_d=din("f1T", [H, FF], BF16),
        f1b_d=din("f1b", [128, FF // 128, 1]),
        f1bh_d=din("f1bh", [128, FF // 128, 1]),
        f2T_d=din("f2T", [128, FF // 128, H], BF16),
        f2b_d=din("f2b", [H, 1]),
        m1Tb_d=din("m1Tb", [H + 1, H2], BF16),
        m1T_d=din("m1T", [H + 1, H2]),
        m1b_d=din("m1b", [H, 2, 1]),
        m2Tb_d=din("m2Tb", [H + 1, 2, H], BF16),
        m2T_d=din("m2T", [H + 1, 2, H]),
        m2b_d=din("m2b", [H, 1]),
        m2wb_d=din("m2wb", [H, H2], BF16),
        wcat_d=din("wcat", [H + 1, WCAT], BF16),
        fbs_d=din("fbs", [TPC, H]),
        wt_d=din("WTc", [NOC, 128, 20, OC], BF16),
    )
    dd["out_d"] = nc.dram_tensor("outf", [H, TQ], F32, kind="ExternalOutput")
    if cfg.get("debug"):
        for nm, shp in [("d_xcf", [H, B, L]), ("d_x1n", [H, TSH]),
                        ("d_xe", [H, TSH]), 
                        ("d_grads", [H + 1, 4 * H]),
                        ("d_q2T", [H, TPC]), ("d_nm1T", [H, H2]),
                        ("d_xff", [H, TPC]), ("d_kf", [HD, NH, B, L]),
                        ("d_h", [TPC, 194]), ("d_dz", [TPC, H2])]:
            dd[nm] = nc.dram_tensor(nm, shp, F32, kind="ExternalOutput")
        for nm, shp in [("d_arin", [TQ, H]), ("d_rs", [TQ, H])]:
            dd[nm] = nc.dram_tensor(nm, shp, BF16, kind="ExternalOutput")

    with tile.TileContext(nc) as tc:
        _body(nc, tc, dd, cfg)
    nc.compile()
    return nc


def _body(nc, tc, dd, cfg):
    from contextlib import ExitStack
    stack = ExitStack()

    def pool(name, bufs, space="SBUF"):
        return stack.enter_context(tc.tile_pool(name=name, bufs=bufs, space=space))

    const = pool("const", 1)
    big = pool("big", 1)
    work = pool("work", 1)
    wstr = pool("wstr", cfg["w_bufs"])
    psxp = pool("psxp", 2, "PSUM")
    pss = pool("pss", 3, "PSUM")
    pscp = pool("pscp", 1, "PSUM")
    dram = pool("dram", 1, "DRAM")
    ar_in = dram.tile([TQ, H], BF16, tag="ar_in", name="ar_in")
    ar_out = dram.tile([TQ, H], BF16, tag="ar_out", name="ar_out")
    gr_in = dram.tile([H + 1, 4 * H], BF16, tag="gr_in", name="gr_in")
    gr_out = dram.tile([H + 1, 4 * H], BF16, tag="gr_out", name="gr_out",
                       addr_space="Shared")

    def ld(dram_t, tag):
        t = const.tile(list(dram_t.shape), dram_t.dtype, tag=tag, name=tag)
        nc.sync.dma_start(t[:], dram_t[:])
        return t

    qwTb = ld(dd["qwTb_d"], "qwTb"); qb = ld(dd["qb_d"], "qb")
    ipqT = ld(dd["ipqT_d"], "ipqT"); ipkT = ld(dd["ipkT_d"], "ipkT")
    ipvT = ld(dd["ipvT_d"], "ipvT")
    ipqb = ld(dd["ipqb_d"], "ipqb"); ipkb = ld(dd["ipkb_d"], "ipkb")
    opT = ld(dd["opT_d"], "opT"); opb = ld(dd["opb_d"], "opb")
    ln1w = ld(dd["ln1w_d"], "ln1w"); ln1b = ld(dd["ln1b_d"], "ln1b")
    ln2w = ld(dd["ln2w_d"], "ln2w"); ln2b = ld(dd["ln2b_d"], "ln2b")
    f1T = ld(dd["f1T_d"], "f1T"); f1b = ld(dd["f1b_d"], "f1b")
    f1bh = ld(dd["f1bh_d"], "f1bh")
    f2T = ld(dd["f2T_d"], "f2T"); f2b = ld(dd["f2b_d"], "f2b")
    m1Tb = ld(dd["m1Tb_d"], "m1Tb"); m1T = ld(dd["m1T_d"], "m1T")
    m1b = ld(dd["m1b_d"], "m1b")
    m2Tb = ld(dd["m2Tb_d"], "m2Tb"); m2T = ld(dd["m2T_d"], "m2T")
    m2b = ld(dd["m2b_d"], "m2b")
    m2wb = ld(dd["m2wb_d"], "m2wb")
    wcat = ld(dd["wcat_d"], "wcat")
    fbs = ld(dd["fbs_d"], "fbs")
    pmT = ld(dd["pmT_d"], "pmT")

    vb_bc = const.tile([128, H], F32, tag="vb_bc", name="vb_bc")
    nc.sync.dma_start(vb_bc[:], dd["ipvb_d"][:].to_broadcast([128, H]))

    ident = const.tile([128, 128], F32, tag="ident", name="ident")
    make_identity(nc, ident[:])
    identb = const.tile([128, 128], BF16, tag="identb", name="identb")
    nc.vector.tensor_copy(identb[:], ident[:])
    ones_col = const.tile([128, 1], F32, tag="ones_col", name="ones_col")
    nc.vector.memset(ones_col[:], 1.0)
    ones_row = const.tile([1, H], F32, tag="ones_row", name="ones_row")
    nc.vector.memset(ones_row[:], 1.0)
    zb = const.tile([128, 1], F32, tag="zb", name="zb")
    nc.vector.memset(zb[:], 0.0)
    eps1 = const.tile([1, 1], F32, tag="eps1", name="eps1")
    nc.vector.memset(eps1[:], 1e-5)

    pid = nc.partition_id()
    qoff = pid * LSH

    # ============ F1: front ============
    xTb = big.tile([H, TQ], BF16, tag="xTb", name="xTb")
    nc.sync.dma_start(xTb[:], dd["xTb_d"][:])

    xcf = big.tile([H, B, L], F32, tag="xcf", name="xcf")
    xcb = big.tile([H, B, L], BF16, tag="xcb", name="xcb")
    nc.vector.tensor_copy(xcf[:, :, 0:PM],
                          pmT[:].unsqueeze(1).to_broadcast([H, B, PM]))
    nc.gpsimd.tensor_copy(xcb[:, :, 0:PM],
                          pmT[:].unsqueeze(1).to_broadcast([H, B, PM]))
    nc.sync.dma_start(xcf[:, :, PM + S:L],
                        dd["xT_d"][:].rearrange("h (b s) -> h b s", b=B))
    nc.gpsimd.tensor_copy(xcb[:, :, PM + S:L],
                          xTb[:].rearrange("h (b s) -> h b s", b=B))

    # neural-memory retrieve for all 768 tokens -> nmm region of xcf
    for c in range(2):
        sl = slice(c * CH, (c + 1) * CH)
        ps = pss.tile([H, CH], F32, tag="ps", name="ps_q1")
        _mm(nc, ps[:], qwTb[:], xTb[:, sl], True, True)
        q1c = work.tile([H, CH], F32, tag="q1c", name="q1c", bufs=2)
        nc.scalar.activation(q1c[:], ps[:], AF.Identity, bias=qb[:])
        sq = work.tile([H, CH], F32, tag="l2sq", name="l2sq", bufs=2)
        nc.vector.tensor_mul(sq[:], q1c[:], q1c[:])
        ps_s = pss.tile([1, CH], F32, tag="ps", name="ps_l2s")
        _mm(nc, ps_s[:], ones_col[:H, :], sq[:], True, True)
        rsq = work.tile([1, CH], F32, tag="l2r", name="l2r", bufs=1)
        _rsqrt(nc, work, "l2", ps_s[:], rsq[:])
        ps_b = pss.tile([H, CH], F32, tag="ps", name="ps_l2b")
        _mm(nc, ps_b[:], ones_row[:], rsq[:], True, True)
        qn = work.tile([H, CH], F32, tag="qn", name="qn", bufs=2)
        nc.vector.tensor_mul(qn[:], q1c[:], ps_b[:])
        qry = work.tile([H, CH], BF16, tag="qry", name="qry", bufs=2)
        nc.scalar.activation(qry[:], qn[:], AF.Silu, bias=zb[:H, :])
        h1 = []
        for m in range(2):
            psm = pss.tile([H, CH], F32, tag="ps", name="ps_h1")
            _mm(nc, psm[:], m1Tb[0:H, m * H:(m + 1) * H], qry[:], True, True)
            h1c = work.tile([H, CH], BF16, tag="h1c", name="h1c", bufs=2)
            nc.scalar.activation(h1c[:], psm[:], AF.Silu, bias=m1b[:, m, :])
            h1.append(h1c)
        ps2 = pss.tile([H, CH], F32, tag="ps", name="ps_nmm")
        _mm(nc, ps2[:], m2Tb[0:H, 0, :], h1[0][:], True, False)
        _mm(nc, ps2[:], m2Tb[0:H, 1, :], h1[1][:], False, True)
        nc.scalar.activation(
            xcf[:, c * 4:(c + 1) * 4, PM:PM + S],
            ps2[:].rearrange("h (b s) -> h b s", b=4), AF.Identity, bias=m2b[:])
        nc.gpsimd.tensor_copy(xcb[:, c * 4:(c + 1) * 4, PM:PM + S],
                              xcf[:, c * 4:(c + 1) * 4, PM:PM + S])

    # k (all tokens), q (own 26 positions)
    kf = big.tile([HD, NH, B, L], BF16, tag="kf", name="kf")
    q_sel = big.tile([HD, NH, B, LSH], BF16, tag="q_sel", name="q_sel")
    xcb_flat = xcb[:].rearrange("h b l -> h (b l)")
    ECH = NTOK // 4
    for hh in range(NH):
        for c in range(4):
            sl = slice(c * ECH, (c + 1) * ECH)
            ps = pss.tile([HD, ECH], F32, tag="ps", name="ps_k")
            _mm(nc, ps[:], ipkT[:, hh, :], xcb_flat[:, sl], True, True)
            nc.scalar.activation(
                kf[:].rearrange("d n b l -> d n (b l)")[:, hh, sl],
                ps[:], AF.Identity, bias=ipkb[:, hh, :])
        psq = pss.tile([HD, TSH], F32, tag="ps", name="ps_q")
        _mm(nc, psq[:], ipqT[:, hh, :], xcb[:, :, ds(qoff, LSH)], True, True)
        nc.scalar.activation(q_sel[:, hh, :, :],
                             psq[:].rearrange("d (b l) -> d b l", b=B),
                             AF.Identity, bias=ipqb[:, hh, :])

    # v token-major per batch: [128, B, H] + [80, B, H] (bf16)
    v_tm0 = big.tile([128, B, H], BF16, tag="v_tm0", name="v_tm0")
    v_tm1 = big.tile([80, B, H], BF16, tag="v_tm1", name="v_tm1")
    for b in range(B):
        for tt, dst, npart in ((0, v_tm0, 128), (1, v_tm1, 80)):
            ps = pss.tile([128, H], F32, tag="ps", name="ps_v")
            toks = slice(tt * 128, tt * 128 + npart)
            _mm(nc, ps[:npart, :], xcb[:, b, toks], ipvT[:], True, True)
            nc.vector.tensor_add(dst[:, b, :], ps[:npart, :], vb_bc[:npart, :])

    # attention: scores/exp batched over pairs of batches per head
    of = big.tile([HD, NH, B, LSH], BF16, tag="of", name="of")
    for hh in range(NH):
        for bg in range(4):
            sc2 = pss.tile([64, L], F32, tag="ps", name="sc2")
            for bq in range(2):
                b = bg * 2 + bq
                _mm(nc, sc2[bq * 32:bq * 32 + LSH, :],
                    q_sel[:, hh, b, :], kf[:, hh, b, :], True, True)
            e2a = work.tile([64, L], BF16, tag="e2a", name="e2a", bufs=2)
            den2 = work.tile([64, 1], F32, tag="den2", name="den2", bufs=2)
            nc.scalar.activation(e2a[:], sc2[:], AF.Exp, bias=zb[:64, :],
                                 accum_out=den2[:])
            rden = work.tile([64, 1], F32, tag="rden", name="rden", bufs=2)
            nc.vector.reciprocal(rden[:], den2[:])
            a2 = work.tile([64, L], BF16, tag="a2", name="a2", bufs=2)
            nc.vector.tensor_scalar_mul(a2[:], e2a[:], rden[:])
            at2 = work.tile([128, 2, 64], BF16, tag="at2", name="at2", bufs=2)
            for kc, npart in ((0, 128), (1, 80)):
                pst = pss.tile([128, 64], BF16, tag="ps", name="ps_t")
                nc.tensor.transpose(pst[:npart, :],
                                    a2[:, kc * 128:kc * 128 + npart],
                                    identb[:64, :64])
                nc.vector.tensor_copy(at2[:npart, kc, :], pst[:npart, :])
            for bq in range(2):
                b = bg * 2 + bq
                ps_o = pss.tile([HD, LSH], F32, tag="ps", name="ps_o")
                for tt, vsrc, npart in ((0, v_tm0, 128), (1, v_tm1, 80)):
                    _mm(nc, ps_o[:], vsrc[:, b, hh * HD:(hh + 1) * HD],
                        at2[:npart, tt, bq * 32:bq * 32 + LSH], tt == 0, tt == 1)
                nc.scalar.copy(of[:, hh, b, :], ps_o[:])

    # out_proj + residual
    ps_op = pss.tile([H, TSH], F32, tag="ps", name="ps_op")
    for hh in range(NH):
        _mm(nc, ps_op[:], opT[:, hh, :],
            of[:, hh, :, :].rearrange("d b l -> d (b l)"), hh == 0, hh == 1)
    x1 = big.tile([H, TSH], F32, tag="x1", name="x1")
    nc.vector.tensor_scalar_add(x1[:], ps_op[:], opb[:])
    nc.vector.tensor_add(x1[:].rearrange("h (b l) -> h b l", b=B),
                         x1[:].rearrange("h (b l) -> h b l", b=B),
                         xcf[:, :, ds(qoff, LSH)])

    x1n = big.tile([H, TSH], F32, tag="x1n", name="x1n")
    _layernorm(nc, pss, work, x1[:], x1n[:], ln1w, ln1b, ones_col, ones_row, eps1)
    x1nb = big.tile([H, TSH], BF16, tag="x1nb", name="x1nb")
    nc.gpsimd.tensor_copy(x1nb[:], x1n[:])

    ps2f = pss.tile([H, TSH], F32, tag="ps", name="ps_ff2")
    for m in range(FF // 128):
        psf = pss.tile([128, TSH], F32, tag="ps", name="ps_ff1")
        _mm(nc, psf[:], f1T[:, m * 128:(m + 1) * 128], x1nb[:], True, True)
        h_ffn = work.tile([128, TSH], BF16, tag="h_ffn", name="h_ffn", bufs=3)
        nc.scalar.activation(h_ffn[:], psf[:], AF.Silu, bias=f1b[:, m, :])
        _mm(nc, ps2f[:], f2T[:, m, :], h_ffn[:], m == 0, m == FF // 128 - 1)
    x2 = big.tile([H, TSH], F32, tag="x2", name="x2")
    nc.vector.tensor_scalar_add(x2[:], ps2f[:], f2b[:])
    nc.vector.tensor_add(x2[:], x2[:], x1n[:])

    e2 = big.tile([H, TSH], F32, tag="e2", name="e2")
    _layernorm(nc, pss, work, x2[:], e2[:], ln2w, ln2b, ones_col, ones_row, eps1)
    # repack xe to K=128 lhsT tiles [128, 20, 8] via a DRAM round-trip:
    # per-batch transpose+silu -> [26, b, 96] -> DRAM rows [8, 2560]
    # (64-el zero pad per batch) -> one strided gather.
    xe_dram = dram.tile([B, 2560], BF16, tag="xe_dram", name="xe_dram")
    xe_tokB = big.tile([LSH, B, H], BF16, tag="xe_tokB", name="xe_tokB")
    e23 = e2[:].rearrange("h (b l) -> h b l", b=B)
    for b in range(B):
        pst = pss.tile([LSH, H], F32, tag="ps", name="ps_xet")
        nc.tensor.transpose(pst[:], e23[:, b, :], ident[:H, :H])
        nc.scalar.activation(xe_tokB[:, b, :], pst[:], AF.Silu, bias=zb[:LSH, :])
    zpad = const.tile([B, 64], BF16, tag="zpad", name="zpad")
    nc.vector.memset(zpad[:], 0.0)
    nc.scalar.dma_start(xe_dram[:, 2496:2560], zpad[:])
    nc.scalar.dma_start(
        xe_dram[:, 0:DK].rearrange("b (l h) -> l b h", h=H), xe_tokB[:])
    xe128 = big.tile([128, B, 20], BF16, tag="xe128", name="xe128")
    nc.scalar.dma_start(
        xe128[:], xe_dram[:, :].rearrange("b (c p) -> p b c", p=128))

    if cfg.get("debug"):
        nc.sync.dma_start(dd["d_xcf"][:], xcf[:])
        nc.sync.dma_start(dd["d_x1n"][:], x1n[:])
        dxe = work.tile([H, TSH], F32, tag="dxe", name="dxe")
        nc.vector.tensor_copy(dxe[:], xeb[:])
        nc.sync.dma_start(dd["d_xe"][:], dxe[:])
        dkf = work.tile([HD, NH, B, L], F32, tag="dkf", name="dkf")
        nc.vector.tensor_copy(dkf[:], kf[:])
        nc.sync.dma_start(dd["d_kf"][:], dkf[:])

    # ============ F2: big matmul (K-sharded) + chunked ReduceScatter ============
    # ar_in rows are (s, b)-major; the chunk DMA writes through a
    # batch-first view so src (SBUF partition=b) and dst dims align.
    ar_in_b = ar_in[:].rearrange("(s b) h -> b s h", b=B)   # [8, 96, 96] strided
    wt4 = dd["wt_d"][:]
    for ci in range(NOC):
        psx = psxp.tile([B, OC], F32, tag="psx", name="psx")
        for k0 in range(0, 20, 2):
            wt = wstr.tile([128, 2, OC], BF16, tag="wt", name="wt")
            nc.sync.dma_start(wt[:], wt4[ci, :, k0:k0 + 2, :])
            for k1 in range(2):
                kg = k0 + k1
                for j0 in range(0, OC, 512):
                    j1 = min(j0 + 512, OC)
                    _mm(nc, psx[:, j0:j1], xe128[:, :, kg], wt[:, k1, j0:j1],
                        kg == 0, kg == 19)
        xfp = work.tile([B, OC], BF16, tag="xfp", name="xfp", bufs=2)
        nc.scalar.copy(xfp[:], psx[:])
        nc.scalar.dma_start(ar_in_b[:, 8 * ci:8 * ci + 8, :],
                            xfp[:].rearrange("b (s h) -> b s h", s=8))
        if ci in (5, 10, 11):
            lo = {5: 0, 10: 384, 11: 704}[ci]
            hi = {5: 384, 10: 704, 11: 768}[ci]
            nc.gpsimd.collective_compute(
                "AllReduce", OP.add,
                replica_groups=[list(range(NC))],
                ins=[ar_in[lo:hi, :].opt()],
                outs=[ar_out[lo:hi, :].opt()],
            )

    # ============ F3: tail (96 tokens per core) ============
    xf_bf = big.tile([TPC, H], BF16, tag="xf_bf", name="xf_bf")
    for g in range(NG):
        nc.scalar.dma_start(xf_bf[24 * g:24 * g + 24, :],
                            ar_out[ds(192 * g + pid * 24, 24), :])
    xf_tm = big.tile([TPC, H], F32, tag="xf_tm", name="xf_tm")
    nc.vector.tensor_add(xf_tm[:], xf_bf[:], fbs[:])

    ps_xt = pss.tile([H, TPC], F32, tag="ps", name="ps_xt")
    nc.tensor.transpose(ps_xt[:], xf_tm[:], ident[:TPC, :TPC])
    xffT = big.tile([H, TPC], F32, tag="xffT", name="xffT")
    nc.vector.tensor_copy(xffT[:], ps_xt[:])
    xffTb = big.tile([H + 1, TPC], BF16, tag="xffTb", name="xffTb")
    nc.vector.tensor_copy(xffTb[0:H, :], ps_xt[:])
    nc.vector.memset(xffTb[H:H + 1, :], 1.0)

    psc = pscp.tile([TPC, WCAT], F32, tag="psc", name="psc")
    _mm(nc, psc[:], xffTb[:], wcat[:], True, True)
    zsl = psc[:, 0:H2]
    kpsl = psc[:, H2:H2 + H + 1]
    vpsl = psc[:, H2 + H + 1:H2 + 2 * H + 1]
    q2sl = psc[:, H2 + 2 * H + 1:WCAT]

    kp_b = big.tile([TPC, H + 1], BF16, tag="kp_b", name="kp_b")
    nc.vector.tensor_copy(kp_b[:], kpsl)
    q2r = big.tile([TPC, H], F32, tag="q2r", name="q2r")
    nc.vector.tensor_copy(q2r[:], q2sl)
    vp_sb = big.tile([TPC, H], F32, tag="vp_sb", name="vp_sb")
    nc.vector.tensor_copy(vp_sb[:], vpsl)

    # h = silu(z), sp = sig(z)*(1 + z - h)   (z includes fused bias)
    thz = work.tile([TPC, H2], F32, tag="thz", name="thz")
    nc.scalar.activation(thz[:], zsl, AF.Tanh, bias=zb[:TPC, :], scale=0.5)
    nc.vector.tensor_scalar(thz[:], thz[:], 0.5, 0.5, OP.mult, OP.add)
    h_tm = big.tile([TPC, 194], BF16, tag="h_tm", name="h_tm")
    nc.vector.tensor_mul(h_tm[:, 0:H], psc[:, 0:H], thz[:, 0:H])
    nc.vector.tensor_mul(h_tm[:, H + 1:2 * H + 1], psc[:, H:H2], thz[:, H:H2])
    nc.vector.memset(h_tm[:, H:H + 1], 1.0)
    nc.vector.memset(h_tm[:, 2 * H + 1:2 * H + 2], 1.0)
    sp = big.tile([TPC, H2], F32, tag="sp", name="sp")
    nc.vector.tensor_scalar(sp[:], thz[:], -1.0, 1.0, OP.mult, OP.add)
    nc.vector.tensor_mul(sp[:], sp[:], zsl)
    nc.vector.tensor_scalar_add(sp[:], sp[:], 1.0)
    nc.vector.tensor_mul(sp[:], sp[:], thz[:])

    # pred via h_fm
    h_fm = big.tile([H, 2, TPC], BF16, tag="h_fm", name="h_fm")
    for m in range(2):
        pst = pss.tile([H, TPC], BF16, tag="ps", name="ps_hf")
        nc.tensor.transpose(pst[:], h_tm[:, m * (H + 1):m * (H + 1) + H],
                            identb[:TPC, :TPC])
        nc.vector.tensor_copy(h_fm[:, m, :], pst[:])
    psp = pss.tile([TPC, H], F32, tag="ps", name="ps_pred")
    _mm(nc, psp[:], h_fm[:, 0, :], m2Tb[0:H, 0, :], True, False)
    _mm(nc, psp[:], h_fm[:, 1, :], m2Tb[0:H, 1, :], False, True)
    dpr_b = big.tile([TPC, H], BF16, tag="dpr_b", name="dpr_b")
    nc.vector.tensor_sub(dpr_b[:], psp[:], vp_sb[:])

    pst2 = pss.tile([H, TPC], BF16, tag="ps", name="ps_dprT")
    nc.tensor.transpose(pst2[:], dpr_b[:], identb[:TPC, :TPC])
    dprT = big.tile([H, TPC], BF16, tag="dprT", name="dprT")
    nc.vector.tensor_copy(dprT[:], pst2[:])

    psd = pss.tile([TPC, H2], F32, tag="ps", name="ps_dz")
    _mm(nc, psd[:], dprT[:], m2wb[:], True, True)
    dz_b = big.tile([TPC, H2], BF16, tag="dz_b", name="dz_b")
    nc.vector.tensor_mul(dz_b[:], psd[:], sp[:])

    # grads: g1 [97, 192] = kp_aug^T dz ; g2 [97, 2, 96] = h_aug^T dpr
    psg1 = pss.tile([H + 1, H2], F32, tag="ps", name="ps_g1")
    _mm(nc, psg1[:], kp_b[:], dz_b[:], True, True)
    psg2 = pss.tile([H + 1, 2, H], F32, tag="ps", name="ps_g2")
    for m in range(2):
        _mm(nc, psg2[:, m, :], h_tm[:, m * (H + 1):(m + 1) * (H + 1)],
            dpr_b[:], True, True)
    grads = big.tile([H + 1, 4 * H], BF16, tag="grads", name="grads")
    nc.vector.tensor_copy(grads[:, 0:H2], psg1[:])
    nc.vector.tensor_copy(grads[:, H2:4 * H],
                          psg2[:].rearrange("p m h -> p (m h)"))
    nc.scalar.dma_start(gr_in[:, :], grads[:])
    nc.gpsimd.collective_compute(
        "AllReduce", OP.add,
        replica_groups=[list(range(NC))],
        ins=[gr_in[:, :].opt()],
        outs=[gr_out[:, :].opt()],
    )

    # q2 = l2norm over features (token-major) then transpose
    sqq = work.tile([TPC, H], F32, tag="sqq", name="sqq")
    nc.vector.tensor_mul(sqq[:], q2r[:], q2r[:])
    ssum = work.tile([TPC, 1], F32, tag="ssum", name="ssum")
    nc.vector.reduce_sum(ssum[:], sqq[:], axis=mybir.AxisListType.X)
    rs2 = work.tile([TPC, 1], F32, tag="rs2", name="rs2")
    _rsqrt(nc, work, "l2t", ssum[:], rs2[:])
    q2n = work.tile([TPC, H], BF16, tag="q2n", name="q2n")
    nc.vector.tensor_scalar_mul(q2n[:], q2r[:], rs2[:])
    ps_qt = pss.tile([H, TPC], BF16, tag="ps", name="ps_q2T")
    nc.tensor.transpose(ps_qt[:], q2n[:], identb[:TPC, :TPC])
    q2T = big.tile([H + 1, TPC], BF16, tag="q2T", name="q2T")
    nc.vector.tensor_copy(q2T[0:H, :], ps_qt[:])
    nc.vector.memset(q2T[H:H + 1, :], 1.0)

    # ============ F4: finalize ============
    grd = big.tile([H + 1, 4 * H], BF16, tag="grd", name="grd")
    nc.scalar.dma_start(grd[:], gr_out[:, :])
    THP = THETA * 2.0 / (TQ * H)

    if cfg.get("debug"):
        nc.sync.dma_start(dd["d_arin"][:], ar_in[:, :])
        nc.sync.dma_start(dd["d_rs"][:], ar_out[:, :])
        nc.sync.dma_start(dd["d_grads"][:], gr_out[:, :])
        nc.sync.dma_start(dd["d_xff"][:], xffT[:])
        dq2 = work.tile([H, TPC], F32, tag="dq2", name="dq2")
        nc.vector.tensor_copy(dq2[:], q2T[:])
        nc.sync.dma_start(dd["d_q2T"][:], dq2[:])
        dh = work.tile([TPC, 194], F32, tag="dh", name="dh")
        nc.vector.tensor_copy(dh[:], h_tm[:])
        nc.sync.dma_start(dd["d_h"][:], dh[:])
        ddz = work.tile([TPC, H2], F32, tag="ddz", name="ddz")
        nc.vector.tensor_copy(ddz[:], dz_b[:])
        nc.sync.dma_start(dd["d_dz"][:], ddz[:])

    nm1T = big.tile([H + 1, H2], F32, tag="nm1T", name="nm1T")
    tg1 = work.tile([H + 1, H2], F32, tag="tg1", name="tg1")
    nc.vector.tensor_scalar_mul(tg1[:], grd[:, 0:H2], THP)
    nc.vector.tensor_scalar_mul(nm1T[:], m1T[:], ALPHA)
    nc.vector.tensor_sub(nm1T[:], nm1T[:], tg1[:])
    nm1Tb = big.tile([H + 1, H2], BF16, tag="nm1Tb", name="nm1Tb")
    nc.vector.tensor_copy(nm1Tb[:], nm1T[:])
    if cfg.get("debug"):
        nc.sync.dma_start(dd["d_nm1T"][:], nm1T[0:H, :])

    nm2T = big.tile([H + 1, 2, H], BF16, tag="nm2T", name="nm2T")
    tg2 = work.tile([H + 1, 2, H], F32, tag="tg2", name="tg2")
    nc.vector.tensor_scalar_mul(tg2[:].rearrange("p m h -> p (m h)"),
                                grd[:, H2:4 * H], THP)
    tg2b = work.tile([H + 1, 2, H], F32, tag="tg2b", name="tg2b")
    nc.vector.tensor_scalar_mul(tg2b[:].rearrange("p m h -> p (m h)"),
                                m2T[:].rearrange("p m h -> p (m h)"), ALPHA)
    nc.vector.tensor_sub(nm2T[:].rearrange("p m h -> p (m h)"),
                         tg2b[:].rearrange("p m h -> p (m h)"),
                         tg2[:].rearrange("p m h -> p (m h)"))
    # chunk-1 bias row must not double-count nm2b
    nc.vector.memset(nm2T[H:H + 1, 1, :], 0.0)

    # retrieve with updated memory for own 96 tokens (biases ride the
    # augmented ones rows)
    uu = []
    for m in range(2):
        psu = pss.tile([H, TPC], F32, tag="ps", name="ps_u")
        _mm(nc, psu[:], nm1Tb[:, m * H:(m + 1) * H], q2T[:], True, True)
        thu = work.tile([H, TPC], F32, tag="thu", name="thu", bufs=2)
        nc.scalar.activation(thu[:], psu[:], AF.Tanh, bias=zb[:H, :], scale=0.5)
        nc.vector.tensor_scalar(thu[:], thu[:], 0.5, 0.5, OP.mult, OP.add)
        u_b = work.tile([H + 1, TPC], BF16, tag="u_b", name="u_b", bufs=2)
        nc.vector.tensor_mul(u_b[0:H, :], psu[:], thu[:])
        nc.vector.memset(u_b[H:H + 1, :], 1.0)
        uu.append(u_b)
    psy = pss.tile([H, TPC], F32, tag="ps", name="ps_y")
    _mm(nc, psy[:], nm2T[:, 0, :], uu[0][:], True, False)
    _mm(nc, psy[:], nm2T[:, 1, :], uu[1][:], False, True)
    thy = work.tile([H, TPC], F32, tag="thy", name="thy")
    nc.scalar.activation(thy[:], psy[:], AF.Tanh, bias=zb[:H, :], scale=0.5)
    nc.vector.tensor_scalar(thy[:], thy[:], 0.5, 0.5, OP.mult, OP.add)
    ot = work.tile([H, TPC], F32, tag="ot", name="ot")
    nc.vector.tensor_mul(ot[:], xffT[:], thy[:])

    out3 = dd["out_d"][:].rearrange("h (b s) -> h b s", b=B)
    ot_b = work.tile([H, B, NG * 3], F32, tag="ot_b", name="ot_b")
    nc.vector.tensor_copy(ot_b[:],
                          ot[:].rearrange("h (g sp b) -> h b (g sp)", g=NG, sp=3))
    for g in range(NG):
        nc.scalar.dma_start(
            out3[:, :, ds(24 * g + pid * 3, 3)],
            ot_b[:, :, 3 * g:3 * g + 3])

    stack.close()


def _layernorm(nc, pss, work, src_ap, dst_ap, w_ap, b_ap, ones_col, ones_row, eps1):
    """dst = LN(src) * w + b over the feature (partition) axis; [96, T] APs."""
    T = src_ap.shape[-1]
    ps_s = pss.tile([1, T], F32, tag="ps", name="ps_lns")
    _mm(nc, ps_s[:], ones_col[:H, :], src_ap, True, True)
    mean = work.tile([1, T], F32, tag="ln_mean", name="ln_mean")
    nc.vector.tensor_scalar_mul(mean[:], ps_s[:], 1.0 / H)
    sq = work.tile([H, T], F32, tag="ln_sq", name="ln_sq")
    nc.vector.tensor_mul(sq[:], src_ap, src_ap)
    ps_q = pss.tile([1, T], F32, tag="ps", name="ps_lnq")
    _mm(nc, ps_q[:], ones_col[:H, :], sq[:], True, True)
    var = work.tile([1, T], F32, tag="ln_var", name="ln_var")
    nc.scalar.activation(var[:], ps_q[:], AF.Identity, bias=eps1[:], scale=1.0 / H)
    m2t = work.tile([1, T], F32, tag="ln_m2", name="ln_m2")
    nc.vector.tensor_mul(m2t[:], mean[:], mean[:])
    nc.vector.tensor_sub(var[:], var[:], m2t[:])
    rstd = work.tile([1, T], F32, tag="ln_rstd", name="ln_rstd")
    _rsqrt(nc, work, "ln", var[:], rstd[:])
    nmr = work.tile([1, T], F32, tag="ln_nmr", name="ln_nmr")
    nc.vector.tensor_mul(nmr[:], mean[:], rstd[:])
    nc.vector.tensor_scalar_mul(nmr[:], nmr[:], -1.0)
    ps_a = pss.tile([H, T], F32, tag="ps", name="ps_lna")
    _mm(nc, ps_a[:], ones_row[:], rstd[:], True, True)
    ps_c = pss.tile([H, T], F32, tag="ps", name="ps_lnc")
    _mm(nc, ps_c[:], ones_row[:], nmr[:], True, True)
    t1 = work.tile([H, T], F32, tag="ln_t1", name="ln_t1")
    nc.vector.tensor_mul(t1[:], src_ap, ps_a[:])
    nc.vector.tensor_add(t1[:], t1[:], ps_c[:])
    nc.vector.tensor_scalar(dst_ap, t1[:], w_ap[:], b_ap[:], OP.mult, OP.add)


def prep_inmaps(inputs, cfg=None):
    cfg = cfg or CFG
    f32 = np.float32
    bf16 = ml_dtypes.bfloat16

    def T(a):
        return np.ascontiguousarray(np.asarray(a, f32).T)

    x = np.asarray(inputs["x"], f32)
    ipw = np.asarray(inputs["in_proj_w"], f32)   # [288, 96]
    ipb = np.asarray(inputs["in_proj_b"], f32)   # [288]
    sc = 1.0 / math.sqrt(HD)
    qw_part = ipw[0:H] * sc
    qb_part = ipb[0:H] * sc
    kw_part = ipw[H:2 * H]
    kb_part = ipb[H:2 * H]
    vw_part = ipw[2 * H:3 * H]
    vb_part = ipb[2 * H:3 * H]

    ipqT = np.ascontiguousarray(qw_part.T.reshape(H, NH, HD))
    ipkT = np.ascontiguousarray(kw_part.T.reshape(H, NH, HD))
    ipqb = np.ascontiguousarray(qb_part.reshape(NH, HD).T.reshape(HD, NH, 1))
    ipkb = np.ascontiguousarray(kb_part.reshape(NH, HD).T.reshape(HD, NH, 1))

    opw = np.asarray(inputs["out_proj_w"], f32)
    opT = np.ascontiguousarray(opw.T.reshape(NH, HD, H).transpose(1, 0, 2))

    f1b = np.asarray(inputs["ff1_b"], f32).reshape(FF // 128, 128, 1)
    f1b = np.ascontiguousarray(f1b.transpose(1, 0, 2))
    f2T = T(inputs["ff2_w"])
    f2T = np.ascontiguousarray(f2T.reshape(FF // 128, 128, H).transpose(1, 0, 2))

    m1w = np.asarray(inputs["m1_w"], f32)        # [192, 96]
    m1bv = np.asarray(inputs["m1_b"], f32)       # [192]
    m2w = np.asarray(inputs["m2_w"], f32)        # [96, 192]
    m2bv = np.asarray(inputs["m2_b"], f32)       # [96]
    kw = np.asarray(inputs["k_w"], f32)
    kb = np.asarray(inputs["k_b"], f32)
    vw = np.asarray(inputs["v_w"], f32)
    vb = np.asarray(inputs["v_b"], f32)
    qw = np.asarray(inputs["q_w"], f32)
    qbv = np.asarray(inputs["q_b"], f32)

    m1b = np.ascontiguousarray(m1bv.reshape(2, H, 1).transpose(1, 0, 2))
    m1T_aug = np.concatenate([T(m1w), m1bv.reshape(1, H2)], 0)        # [97, 192]
    m2T3 = np.ascontiguousarray(T(m2w).reshape(2, H, H).transpose(1, 0, 2))
    m2T_aug = np.zeros((H + 1, 2, H), f32)
    m2T_aug[0:H] = m2T3
    m2T_aug[H, 0] = m2bv                                              # chunk-0 bias row

    # Wcat [97, 481]: z | kp(+ones col) | vp' | q2
    wcat = np.zeros((H + 1, WCAT), f32)
    m1kw = m1w @ kw                               # [192, 96]
    wcat[0:H, 0:H2] = m1kw.T
    wcat[H, 0:H2] = m1w @ kb + m1bv
    wcat[0:H, H2:H2 + H] = kw.T
    wcat[H, H2:H2 + H] = kb
    wcat[H, H2 + H] = 1.0                         # ones column for g1 bias row
    wcat[0:H, H2 + H + 1:H2 + 2 * H + 1] = vw.T
    wcat[H, H2 + H + 1:H2 + 2 * H + 1] = vb - m2bv
    wcat[0:H, H2 + 2 * H + 1:WCAT] = qw.T
    wcat[H, H2 + 2 * H + 1:WCAT] = qbv

    fwT = np.ascontiguousarray(np.asarray(inputs["final_w"], f32).T)
    fb = np.asarray(inputs["final_b"], f32).reshape(S, H)

    col = lambda k: np.ascontiguousarray(np.asarray(inputs[k], f32).reshape(-1, 1))
    xTf = T(x.reshape(TQ, H))
    base = dict(
        xT=xTf, xTb=xTf.astype(bf16),
        pmT=T(inputs["persistent_memory"]),
        qwTb=T(qw).astype(bf16), qb=col("q_b"),
        ipqT=ipqT.astype(bf16), ipkT=ipkT.astype(bf16),
        ipvT=np.ascontiguousarray(vw_part.T).astype(bf16),
        ipqb=ipqb, ipkb=ipkb,
        ipvb=np.ascontiguousarray(vb_part.reshape(1, H)),
        opT=opT.astype(bf16), opb=col("out_proj_b"),
        ln1w=col("ln1_w"), ln1b=col("ln1_b"),
        ln2w=col("ln2_w"), ln2b=col("ln2_b"),
        f1T=T(inputs["ff1_w"]).astype(bf16), f1b=f1b,
        f1bh=np.ascontiguousarray(f1b * 0.5),
        f2T=f2T.astype(bf16), f2b=col("ff2_b"),
        m1Tb=m1T_aug.astype(bf16), m1T=m1T_aug,
        m1b=m1b,
        m2Tb=m2T_aug.astype(bf16), m2T=m2T_aug,
        m2b=col("m2_b"),
        m2wb=np.ascontiguousarray(m2w).astype(bf16),
        wcat=wcat.astype(bf16),
    )
    in_maps = []
    for c in range(NC):
        m = dict(base)
        shard = fwT[c * DK:(c + 1) * DK]                     # [(l h), 9216]
        shard_pad = np.concatenate([shard, np.zeros((64, DOUT), f32)], 0)
        packed = shard_pad.reshape(20, 128, NOC, OC).transpose(2, 1, 0, 3)
        m["WTc"] = np.ascontiguousarray(packed.astype(bf16))
        # fbs: final_b rows for this core's tokens in (g, sp, b) order
        ss = np.array([24 * g + 3 * c + d for g in range(NG) for d in range(3)])
        m["fbs"] = np.ascontiguousarray(
            np.repeat(fb[ss], B, axis=0))                    # [96, 96]
        in_maps.append(m)
    return in_maps


def get_nc(cfg=None):
    cfg = cfg or CFG
    key = tuple(sorted((k, str(v)) for k, v in cfg.items()))
    if key not in _CACHE:
        _CACHE[key] = build(cfg)
    return _CACHE[key]


def assemble(results):
    """Gather per-core output column slices into the full [B, S, H] output."""
    full = np.zeros((H, TQ), np.float32)
    for c in range(NC):
        outc = results[c]["outf"]                            # [96, 768]
        cols = np.array([b * S + 24 * g + 3 * c + d
                         for g in range(NG) for d in range(3) for b in range(B)])
        full[:, cols] = outc[:, cols]
    return np.ascontiguousarray(full.T).reshape(B, S, H)


def kernel(**inputs):
    nc = get_nc()
    in_maps = prep_inmaps(inputs)
    res = bass_utils.run_bass_kernel_spmd(
        nc, in_maps, core_ids=list(range(NC)), trace=False
    )
    return assemble(res.results)


if __name__ == "__main__":
    print("building...")
    get_nc()
    print("built")


# revision 28
# speedup vs baseline: 1.1247x; 1.0157x over previous
"""Trainium2 Bass kernel for nn_MACTitanLayer (MAC Titan layer, 8 cores).

Structure (v2):
  - Position-sharded encoder front (26 of 208 positions per core), bf16
    matmul operands, one activation table set (exp/tanh only; silu and
    sigmoid are built from tanh, inverse sqrt is Quake-init Newton on the
    vector engine).
  - K-sharded final matmul: core c owns contraction rows for its 26
    positions, streams its [12, 96, 26, 768] bf16 weight shard from HBM
    with a deep prefetch ring that starts at t=0.
  - The partial xf [768, 96] is combined with 4 chunked ReduceScatters
    (s-major row blocks) overlapped with the weight-stream matmuls; each
    core receives only its 96 tail tokens.
  - Token-sharded TTT tail (96 tokens/core): fused projection matmul
    (z|kp|vp|q2 in one 481-wide rhs), gradient partials via ones-column
    tricks, one small grad AllReduce, replicated param update, per-core
    retrieve; the host gathers the 8 output slices.
"""

import math

import numpy as np
import ml_dtypes

import concourse.bass as bass
import concourse.mybir as mybir
import concourse.tile as tile
from concourse import bacc
from concourse import bass_utils
from concourse.bass import ds
from concourse.masks import make_identity

F32 = mybir.dt.float32
BF16 = mybir.dt.bfloat16
I32 = mybir.dt.int32
AF = mybir.ActivationFunctionType
OP = mybir.AluOpType

B, S, H, PM, FF, NH = 8, 96, 96, 16, 2048, 2
ALPHA, THETA = 0.999, 0.3
L = PM + 2 * S            # 208 encoder tokens per batch
NC = 8
LSH = L // NC             # 26 positions per core
DK = LSH * H              # 2496 contraction rows per core
DOUT = S * H              # 9216
TQ = B * S                # 768 tokens
HD = H // NH              # 48
NTOK = B * L              # 1664
TSH = B * LSH             # 208 sharded tokens per core
CH = TQ // 2              # 384
OC = 768                  # big-matmul output chunk (8 s-positions)
NOC = DOUT // OC          # 12
NG = 4                    # ReduceScatter groups (3 chunks each)
TPC = TQ // NC            # 96 tail tokens per core
H2 = 2 * H                # 192
WCAT = H2 + (H + 1) + H + H  # 481: z | kp+ones | vp' | q2

CFG = {"w_bufs": 36, "ll2": 2}

_CACHE = {}

MAGIC = 0x5F3759DF


def _mm(nc, out, lhsT, rhs, start, stop):
    nc.tensor.matmul(out, lhsT, rhs, start=start, stop=stop)


def _rsqrt(nc, work, pool_tag, s_ap, out_ap, iters=2):
    """out = 1/sqrt(s) via Quake init + Newton, all on the vector engine.

    s_ap: f32 AP [P, T] (strictly positive). out_ap: f32 AP same shape.
    """
    shp = list(s_ap.shape)
    y = work.tile(shp, F32, tag=f"{pool_tag}_y", name="rs_y", bufs=1)
    t = work.tile(shp, F32, tag=f"{pool_tag}_t", name="rs_t", bufs=1)
    s = work.tile(shp, F32, tag=f"{pool_tag}_s", name="rs_s", bufs=1)
    nc.vector.tensor_copy(s[:], s_ap)
    # y0 bits = MAGIC - (bits(s) >> 1)  ==  ((bits(s)>>1) ^ -1) + (MAGIC+1)
    nc.vector.tensor_scalar(y[:].bitcast(I32), s[:].bitcast(I32),
                            1, None, OP.logical_shift_right)
    nc.vector.tensor_scalar(y[:].bitcast(I32), y[:].bitcast(I32),
                            -1, None, OP.bitwise_xor)
    nc.vector.tensor_scalar(y[:].bitcast(I32), y[:].bitcast(I32),
                            MAGIC + 1, None, OP.add)
    for it in range(iters):
        nc.vector.tensor_mul(t[:], s[:], y[:])
        nc.vector.tensor_mul(t[:], t[:], y[:])
        nc.vector.tensor_scalar(t[:], t[:], -0.5, 1.5, OP.mult, OP.add)
        nc.vector.tensor_mul(out_ap if it == iters - 1 else y[:], y[:], t[:])


def build(cfg):
    nc = bacc.Bacc("TRN2", target_bir_lowering=False, debug=False, num_devices=NC)

    def din(name, shape, dt=F32):
        return nc.dram_tensor(name, shape, dt, kind="ExternalInput")

    dd = dict(
        xT_d=din("xT", [H, TQ]),
        xTb_d=din("xTb", [H, TQ], BF16),
        pmT_d=din("pmT", [H, PM]),
        qwTb_d=din("qwTb", [H, H], BF16),
        qb_d=din("qb", [H, 1]),
        ipqT_d=din("ipqT", [H, NH, HD], BF16),
        ipkT_d=din("ipkT", [H, NH, HD], BF16),
        ipvT_d=din("ipvT", [H, H], BF16),
        ipqb_d=din("ipqb", [HD, NH, 1]),
        ipkb_d=din("ipkb", [HD, NH, 1]),
        ipvb_d=din("ipvb", [1, H]),
        opT_d=din("opT", [HD, NH, H], BF16),
        opb_d=din("opb", [H, 1]),
        ln1w_d=din("ln1w", [H, 1]), ln1b_d=din("ln1b", [H, 1]),
        ln2w_d=din("ln2w", [H, 1]), ln2b_d=din("ln2b", [H, 1]),
        f1T_d=din("f1T", [H, FF], BF16),
        f1b_d=din("f1b", [128, FF // 128, 1]),
        f1bh_d=din("f1bh", [128, FF // 128, 1]),
        f2T_d=din("f2T", [128, FF // 128, H], BF16),
        f2b_d=din("f2b", [H, 1]),
        m1Tb_d=din("m1Tb", [H + 1, H2], BF16),
        m1T_d=din("m1T", [H + 1, H2]),
        m1b_d=din("m1b", [H, 2, 1]),
        m2Tb_d=din("m2Tb", [H + 1, 2, H], BF16),
        m2T_d=din("m2T", [H + 1, 2, H]),
        m2b_d=din("m2b", [H, 1]),
        m2wb_d=din("m2wb", [H, H2], BF16),
        wcat_d=din("wcat", [H + 1, WCAT], BF16),
        fbs_d=din("fbs", [TPC, H]),
        wt_d=din("WTc", [NOC, 128, 20, OC], BF16),
    )
    dd["out_d"] = nc.dram_tensor("outf", [H, TQ], F32, kind="ExternalOutput")
    if cfg.get("debug"):
        for nm, shp in [("d_xcf", [H, B, L]), ("d_x1n", [H, TSH]),
                        ("d_xe", [H, TSH]), 
                        ("d_grads", [H + 1, 4 * H]),
                        ("d_q2T", [H, TPC]), ("d_nm1T", [H, H2]),
                        ("d_xff", [H, TPC]), ("d_kf", [HD, NH, B, L]),
                        ("d_h", [TPC, 194]), ("d_dz", [TPC, H2])]:
            dd[nm] = nc.dram_tensor(nm, shp, F32, kind="ExternalOutput")
        for nm, shp in [("d_arin", [TQ, H]), ("d_rs", [TQ, H])]:
            dd[nm] = nc.dram_tensor(nm, shp, BF16, kind="ExternalOutput")

    with tile.TileContext(nc) as tc:
        _body(nc, tc, dd, cfg)
    nc.compile()
    return nc


def _body(nc, tc, dd, cfg):
    from contextlib import ExitStack
    stack = ExitStack()

    def pool(name, bufs, space="SBUF"):
        return stack.enter_context(tc.tile_pool(name=name, bufs=bufs, space=space))

    const = pool("const", 1)
    big = pool("big", 1)
    work = pool("work", 1)
    wstr = pool("wstr", cfg["w_bufs"])
    psxp = pool("psxp", 2, "PSUM")
    pss = pool("pss", 3, "PSUM")
    pscp = pool("pscp", 1, "PSUM")
    dram = pool("dram", 1, "DRAM")

    fp_ctr = [0]

    def fps(shape, name):
        fp_ctr[0] += 1
        if fp_ctr[0] % 2 == 0:
            return psxp.tile(shape, F32, tag="psx", name=name)
        return pss.tile(shape, F32, tag="ps", name=name)
    ar_in = dram.tile([TQ, H], BF16, tag="ar_in", name="ar_in")
    ar_out = dram.tile([TQ, H], BF16, tag="ar_out", name="ar_out")
    gr_in = dram.tile([H + 1, 4 * H], BF16, tag="gr_in", name="gr_in")
    gr_out = dram.tile([H + 1, 4 * H], BF16, tag="gr_out", name="gr_out",
                       addr_space="Shared")

    def ld(dram_t, tag):
        t = const.tile(list(dram_t.shape), dram_t.dtype, tag=tag, name=tag)
        nc.sync.dma_start(t[:], dram_t[:])
        return t

    qwTb = ld(dd["qwTb_d"], "qwTb"); qb = ld(dd["qb_d"], "qb")
    ipqT = ld(dd["ipqT_d"], "ipqT"); ipkT = ld(dd["ipkT_d"], "ipkT")
    ipvT = ld(dd["ipvT_d"], "ipvT")
    ipqb = ld(dd["ipqb_d"], "ipqb"); ipkb = ld(dd["ipkb_d"], "ipkb")
    opT = ld(dd["opT_d"], "opT"); opb = ld(dd["opb_d"], "opb")
    ln1w = ld(dd["ln1w_d"], "ln1w"); ln1b = ld(dd["ln1b_d"], "ln1b")
    ln2w = ld(dd["ln2w_d"], "ln2w"); ln2b = ld(dd["ln2b_d"], "ln2b")
    f1T = ld(dd["f1T_d"], "f1T"); f1b = ld(dd["f1b_d"], "f1b")
    f1bh = ld(dd["f1bh_d"], "f1bh")
    f2T = ld(dd["f2T_d"], "f2T"); f2b = ld(dd["f2b_d"], "f2b")
    m1Tb = ld(dd["m1Tb_d"], "m1Tb"); m1T = ld(dd["m1T_d"], "m1T")
    m1b = ld(dd["m1b_d"], "m1b")
    m2Tb = ld(dd["m2Tb_d"], "m2Tb"); m2T = ld(dd["m2T_d"], "m2T")
    m2b = ld(dd["m2b_d"], "m2b")
    m2wb = ld(dd["m2wb_d"], "m2wb")
    wcat = ld(dd["wcat_d"], "wcat")
    fbs = ld(dd["fbs_d"], "fbs")
    pmT = ld(dd["pmT_d"], "pmT")

    vb_bc = const.tile([128, H], F32, tag="vb_bc", name="vb_bc")
    nc.sync.dma_start(vb_bc[:], dd["ipvb_d"][:].to_broadcast([128, H]))

    ident = const.tile([128, 128], F32, tag="ident", name="ident")
    make_identity(nc, ident[:])
    identb = const.tile([128, 128], BF16, tag="identb", name="identb")
    nc.vector.tensor_copy(identb[:], ident[:])
    ones_col = const.tile([128, 1], F32, tag="ones_col", name="ones_col")
    nc.vector.memset(ones_col[:], 1.0)
    ones_row = const.tile([1, H], F32, tag="ones_row", name="ones_row")
    nc.vector.memset(ones_row[:], 1.0)
    zb = const.tile([128, 1], F32, tag="zb", name="zb")
    nc.vector.memset(zb[:], 0.0)
    eps1 = const.tile([1, 1], F32, tag="eps1", name="eps1")
    nc.vector.memset(eps1[:], 1e-5)

    pid = nc.partition_id()
    qoff = pid * LSH

    # ============ F1: front ============
    xTb = big.tile([H, TQ], BF16, tag="xTb", name="xTb")
    nc.sync.dma_start(xTb[:], dd["xTb_d"][:])

    xcf = big.tile([H, B, L], F32, tag="xcf", name="xcf")
    xcb = big.tile([H, B, L], BF16, tag="xcb", name="xcb")
    nc.vector.tensor_copy(xcf[:, :, 0:PM],
                          pmT[:].unsqueeze(1).to_broadcast([H, B, PM]))
    nc.gpsimd.tensor_copy(xcb[:, :, 0:PM],
                          pmT[:].unsqueeze(1).to_broadcast([H, B, PM]))
    nc.sync.dma_start(xcf[:, :, PM + S:L],
                        dd["xT_d"][:].rearrange("h (b s) -> h b s", b=B))
    nc.gpsimd.tensor_copy(xcb[:, :, PM + S:L],
                          xTb[:].rearrange("h (b s) -> h b s", b=B))

    # neural-memory retrieve for all 768 tokens -> nmm region of xcf
    for c in range(2):
        sl = slice(c * CH, (c + 1) * CH)
        ps = fps([H, CH], "ps_q1")
        _mm(nc, ps[:], qwTb[:], xTb[:, sl], True, True)
        q1c = work.tile([H, CH], F32, tag="q1c", name="q1c", bufs=2)
        nc.scalar.activation(q1c[:], ps[:], AF.Identity, bias=qb[:])
        sq = work.tile([H, CH], F32, tag="l2sq", name="l2sq", bufs=2)
        nc.vector.tensor_mul(sq[:], q1c[:], q1c[:])
        ps_s = fps([1, CH], "ps_l2s")
        _mm(nc, ps_s[:], ones_col[:H, :], sq[:], True, True)
        rsq = work.tile([1, CH], F32, tag="l2r", name="l2r", bufs=1)
        _rsqrt(nc, work, "l2", ps_s[:], rsq[:])
        ps_b = fps([H, CH], "ps_l2b")
        _mm(nc, ps_b[:], ones_row[:], rsq[:], True, True)
        qn = work.tile([H, CH], F32, tag="qn", name="qn", bufs=2)
        nc.vector.tensor_mul(qn[:], q1c[:], ps_b[:])
        qry = work.tile([H, CH], BF16, tag="qry", name="qry", bufs=2)
        nc.scalar.activation(qry[:], qn[:], AF.Silu, bias=zb[:H, :])
        h1 = []
        for m in range(2):
            psm = fps([H, CH], "ps_h1")
            _mm(nc, psm[:], m1Tb[0:H, m * H:(m + 1) * H], qry[:], True, True)
            h1c = work.tile([H, CH], BF16, tag="h1c", name="h1c", bufs=2)
            nc.scalar.activation(h1c[:], psm[:], AF.Silu, bias=m1b[:, m, :])
            h1.append(h1c)
        ps2 = fps([H, CH], "ps_nmm")
        _mm(nc, ps2[:], m2Tb[0:H, 0, :], h1[0][:], True, False)
        _mm(nc, ps2[:], m2Tb[0:H, 1, :], h1[1][:], False, True)
        nc.scalar.activation(
            xcf[:, c * 4:(c + 1) * 4, PM:PM + S],
            ps2[:].rearrange("h (b s) -> h b s", b=4), AF.Identity, bias=m2b[:])
        nc.gpsimd.tensor_copy(xcb[:, c * 4:(c + 1) * 4, PM:PM + S],
                              xcf[:, c * 4:(c + 1) * 4, PM:PM + S])

    # k (all tokens), q (own 26 positions)
    kf = big.tile([HD, NH, B, L], BF16, tag="kf", name="kf")
    q_sel = big.tile([HD, NH, B, LSH], BF16, tag="q_sel", name="q_sel")
    xcb_flat = xcb[:].rearrange("h b l -> h (b l)")
    ECH = NTOK // 4
    for hh in range(NH):
        for c in range(4):
            sl = slice(c * ECH, (c + 1) * ECH)
            ps = fps([HD, ECH], "ps_k")
            _mm(nc, ps[:], ipkT[:, hh, :], xcb_flat[:, sl], True, True)
            nc.scalar.activation(
                kf[:].rearrange("d n b l -> d n (b l)")[:, hh, sl],
                ps[:], AF.Identity, bias=ipkb[:, hh, :])
        psq = fps([HD, TSH], "ps_q")
        _mm(nc, psq[:], ipqT[:, hh, :], xcb[:, :, ds(qoff, LSH)], True, True)
        nc.scalar.activation(q_sel[:, hh, :, :],
                             psq[:].rearrange("d (b l) -> d b l", b=B),
                             AF.Identity, bias=ipqb[:, hh, :])

    # v token-major per batch: [128, B, H] + [80, B, H] (bf16)
    v_tm0 = big.tile([128, B, H], BF16, tag="v_tm0", name="v_tm0")
    v_tm1 = big.tile([80, B, H], BF16, tag="v_tm1", name="v_tm1")
    for b in range(B):
        for tt, dst, npart in ((0, v_tm0, 128), (1, v_tm1, 80)):
            ps = fps([128, H], "ps_v")
            toks = slice(tt * 128, tt * 128 + npart)
            _mm(nc, ps[:npart, :], xcb[:, b, toks], ipvT[:], True, True)
            nc.vector.tensor_add(dst[:, b, :], ps[:npart, :], vb_bc[:npart, :])

    # attention: scores/exp batched over pairs of batches per head
    of = big.tile([HD, NH, B, LSH], BF16, tag="of", name="of")
    for hh in range(NH):
        for bg in range(4):
            sc2 = fps([64, L], "sc2")
            for bq in range(2):
                b = bg * 2 + bq
                _mm(nc, sc2[bq * 32:bq * 32 + LSH, :],
                    q_sel[:, hh, b, :], kf[:, hh, b, :], True, True)
            e2a = work.tile([64, L], BF16, tag="e2a", name="e2a", bufs=2)
            den2 = work.tile([64, 1], F32, tag="den2", name="den2", bufs=2)
            nc.scalar.activation(e2a[:], sc2[:], AF.Exp, bias=zb[:64, :],
                                 accum_out=den2[:])
            rden = work.tile([64, 1], F32, tag="rden", name="rden", bufs=2)
            nc.vector.reciprocal(rden[:], den2[:])
            a2 = work.tile([64, L], BF16, tag="a2", name="a2", bufs=2)
            nc.vector.tensor_scalar_mul(a2[:], e2a[:], rden[:])
            at2 = work.tile([128, 2, 64], BF16, tag="at2", name="at2", bufs=2)
            for kc, npart in ((0, 128), (1, 80)):
                pst = pss.tile([128, 64], BF16, tag="ps", name="ps_t")
                nc.tensor.transpose(pst[:npart, :],
                                    a2[:, kc * 128:kc * 128 + npart],
                                    identb[:64, :64])
                nc.vector.tensor_copy(at2[:npart, kc, :], pst[:npart, :])
            for bq in range(2):
                b = bg * 2 + bq
                ps_o = fps([HD, LSH], "ps_o")
                for tt, vsrc, npart in ((0, v_tm0, 128), (1, v_tm1, 80)):
                    _mm(nc, ps_o[:], vsrc[:, b, hh * HD:(hh + 1) * HD],
                        at2[:npart, tt, bq * 32:bq * 32 + LSH], tt == 0, tt == 1)
                nc.scalar.copy(of[:, hh, b, :], ps_o[:])

    # out_proj + residual
    ps_op = pss.tile([H, TSH], F32, tag="ps", name="ps_op")
    for hh in range(NH):
        _mm(nc, ps_op[:], opT[:, hh, :],
            of[:, hh, :, :].rearrange("d b l -> d (b l)"), hh == 0, hh == 1)
    x1 = big.tile([H, TSH], F32, tag="x1", name="x1")
    nc.vector.tensor_scalar_add(x1[:], ps_op[:], opb[:])
    nc.vector.tensor_add(x1[:].rearrange("h (b l) -> h b l", b=B),
                         x1[:].rearrange("h (b l) -> h b l", b=B),
                         xcf[:, :, ds(qoff, LSH)])

    x1n = big.tile([H, TSH], F32, tag="x1n", name="x1n")
    _layernorm(nc, pss, work, x1[:], x1n[:], ln1w, ln1b, ones_col, ones_row, eps1)
    x1nb = big.tile([H, TSH], BF16, tag="x1nb", name="x1nb")
    nc.gpsimd.tensor_copy(x1nb[:], x1n[:])

    ps2f = pss.tile([H, TSH], F32, tag="ps", name="ps_ff2")
    for m in range(FF // 128):
        psf = fps([128, TSH], "ps_ff1")
        _mm(nc, psf[:], f1T[:, m * 128:(m + 1) * 128], x1nb[:], True, True)
        h_ffn = work.tile([128, TSH], BF16, tag="h_ffn", name="h_ffn", bufs=3)
        nc.scalar.activation(h_ffn[:], psf[:], AF.Silu, bias=f1b[:, m, :])
        _mm(nc, ps2f[:], f2T[:, m, :], h_ffn[:], m == 0, m == FF // 128 - 1)
    x2 = big.tile([H, TSH], F32, tag="x2", name="x2")
    nc.vector.tensor_scalar_add(x2[:], ps2f[:], f2b[:])
    nc.vector.tensor_add(x2[:], x2[:], x1n[:])

    e2 = big.tile([H, TSH], F32, tag="e2", name="e2")
    _layernorm(nc, pss, work, x2[:], e2[:], ln2w, ln2b, ones_col, ones_row, eps1)
    # repack xe to K=128 lhsT tiles [128, 20, 8] via a DRAM round-trip:
    # per-batch transpose+silu -> [26, b, 96] -> DRAM rows [8, 2560]
    # (64-el zero pad per batch) -> one strided gather.
    xe_dram = dram.tile([B, 2560], BF16, tag="xe_dram", name="xe_dram")
    xe_tokB = big.tile([LSH, B, H], BF16, tag="xe_tokB", name="xe_tokB")
    e23 = e2[:].rearrange("h (b l) -> h b l", b=B)
    for b in range(B):
        pst = pss.tile([LSH, H], F32, tag="ps", name="ps_xet")
        nc.tensor.transpose(pst[:], e23[:, b, :], ident[:H, :H])
        nc.scalar.activation(xe_tokB[:, b, :], pst[:], AF.Silu, bias=zb[:LSH, :])
    zpad = const.tile([B, 64], BF16, tag="zpad", name="zpad")
    nc.vector.memset(zpad[:], 0.0)
    nc.scalar.dma_start(xe_dram[:, 2496:2560], zpad[:])
    nc.scalar.dma_start(
        xe_dram[:, 0:DK].rearrange("b (l h) -> l b h", h=H), xe_tokB[:])
    xe128 = big.tile([128, B, 20], BF16, tag="xe128", name="xe128")
    nc.scalar.dma_start(
        xe128[:], xe_dram[:, :].rearrange("b (c p) -> p b c", p=128))

    if cfg.get("debug"):
        nc.sync.dma_start(dd["d_xcf"][:], xcf[:])
        nc.sync.dma_start(dd["d_x1n"][:], x1n[:])
        dxe = work.tile([H, TSH], F32, tag="dxe", name="dxe")
        nc.vector.tensor_copy(dxe[:], xeb[:])
        nc.sync.dma_start(dd["d_xe"][:], dxe[:])
        dkf = work.tile([HD, NH, B, L], F32, tag="dkf", name="dkf")
        nc.vector.tensor_copy(dkf[:], kf[:])
        nc.sync.dma_start(dd["d_kf"][:], dkf[:])

    # ============ F2: big matmul (K-sharded) + chunked ReduceScatter ============
    # ar_in rows are (s, b)-major; the chunk DMA writes through a
    # batch-first view so src (SBUF partition=b) and dst dims align.
    ar_in_b = ar_in[:].rearrange("(s b) h -> b s h", b=B)   # [8, 96, 96] strided
    wt4 = dd["wt_d"][:]
    for ci in range(NOC):
        psx = psxp.tile([B, OC], F32, tag="psx", name="psx")
        for k0 in range(0, 20, 2):
            wt = wstr.tile([128, 2, OC], BF16, tag="wt", name="wt")
            nc.sync.dma_start(wt[:], wt4[ci, :, k0:k0 + 2, :])
            for k1 in range(2):
                kg = k0 + k1
                for j0 in range(0, OC, 512):
                    j1 = min(j0 + 512, OC)
                    _mm(nc, psx[:, j0:j1], xe128[:, :, kg], wt[:, k1, j0:j1],
                        kg == 0, kg == 19)
        xfp = work.tile([B, OC], BF16, tag="xfp", name="xfp", bufs=2)
        nc.scalar.copy(xfp[:], psx[:])
        nc.scalar.dma_start(ar_in_b[:, 8 * ci:8 * ci + 8, :],
                            xfp[:].rearrange("b (s h) -> b s h", s=8))
        if ci in (5, 10, 11):
            lo = {5: 0, 10: 384, 11: 704}[ci]
            hi = {5: 384, 10: 704, 11: 768}[ci]
            nc.gpsimd.collective_compute(
                "AllReduce", OP.add,
                replica_groups=[list(range(NC))],
                ins=[ar_in[lo:hi, :].opt()],
                outs=[ar_out[lo:hi, :].opt()],
            )

    # ============ F3: tail (96 tokens per core) ============
    xf_bf = big.tile([TPC, H], BF16, tag="xf_bf", name="xf_bf")
    for g in range(NG):
        nc.scalar.dma_start(xf_bf[24 * g:24 * g + 24, :],
                            ar_out[ds(192 * g + pid * 24, 24), :])
    xf_tm = big.tile([TPC, H], F32, tag="xf_tm", name="xf_tm")
    nc.vector.tensor_add(xf_tm[:], xf_bf[:], fbs[:])

    ps_xt = pss.tile([H, TPC], F32, tag="ps", name="ps_xt")
    nc.tensor.transpose(ps_xt[:], xf_tm[:], ident[:TPC, :TPC])
    xffT = big.tile([H, TPC], F32, tag="xffT", name="xffT")
    nc.vector.tensor_copy(xffT[:], ps_xt[:])
    xffTb = big.tile([H + 1, TPC], BF16, tag="xffTb", name="xffTb")
    nc.vector.tensor_copy(xffTb[0:H, :], ps_xt[:])
    nc.vector.memset(xffTb[H:H + 1, :], 1.0)

    psc = pscp.tile([TPC, WCAT], F32, tag="psc", name="psc")
    _mm(nc, psc[:], xffTb[:], wcat[:], True, True)
    zsl = psc[:, 0:H2]
    kpsl = psc[:, H2:H2 + H + 1]
    vpsl = psc[:, H2 + H + 1:H2 + 2 * H + 1]
    q2sl = psc[:, H2 + 2 * H + 1:WCAT]

    kp_b = big.tile([TPC, H + 1], BF16, tag="kp_b", name="kp_b")
    nc.vector.tensor_copy(kp_b[:], kpsl)
    q2r = big.tile([TPC, H], F32, tag="q2r", name="q2r")
    nc.vector.tensor_copy(q2r[:], q2sl)
    vp_sb = big.tile([TPC, H], F32, tag="vp_sb", name="vp_sb")
    nc.vector.tensor_copy(vp_sb[:], vpsl)

    # h = silu(z), sp = sig(z)*(1 + z - h)   (z includes fused bias)
    thz = work.tile([TPC, H2], F32, tag="thz", name="thz")
    nc.scalar.activation(thz[:], zsl, AF.Tanh, bias=zb[:TPC, :], scale=0.5)
    nc.vector.tensor_scalar(thz[:], thz[:], 0.5, 0.5, OP.mult, OP.add)
    h_tm = big.tile([TPC, 194], BF16, tag="h_tm", name="h_tm")
    nc.vector.tensor_mul(h_tm[:, 0:H], psc[:, 0:H], thz[:, 0:H])
    nc.vector.tensor_mul(h_tm[:, H + 1:2 * H + 1], psc[:, H:H2], thz[:, H:H2])
    nc.vector.memset(h_tm[:, H:H + 1], 1.0)
    nc.vector.memset(h_tm[:, 2 * H + 1:2 * H + 2], 1.0)
    sp = big.tile([TPC, H2], F32, tag="sp", name="sp")
    nc.vector.tensor_scalar(sp[:], thz[:], -1.0, 1.0, OP.mult, OP.add)
    nc.vector.tensor_mul(sp[:], sp[:], zsl)
    nc.vector.tensor_scalar_add(sp[:], sp[:], 1.0)
    nc.vector.tensor_mul(sp[:], sp[:], thz[:])

    # pred via h_fm
    h_fm = big.tile([H, 2, TPC], BF16, tag="h_fm", name="h_fm")
    for m in range(2):
        pst = pss.tile([H, TPC], BF16, tag="ps", name="ps_hf")
        nc.tensor.transpose(pst[:], h_tm[:, m * (H + 1):m * (H + 1) + H],
                            identb[:TPC, :TPC])
        nc.vector.tensor_copy(h_fm[:, m, :], pst[:])
    psp = pss.tile([TPC, H], F32, tag="ps", name="ps_pred")
    _mm(nc, psp[:], h_fm[:, 0, :], m2Tb[0:H, 0, :], True, False)
    _mm(nc, psp[:], h_fm[:, 1, :], m2Tb[0:H, 1, :], False, True)
    dpr_b = big.tile([TPC, H], BF16, tag="dpr_b", name="dpr_b")
    nc.vector.tensor_sub(dpr_b[:], psp[:], vp_sb[:])

    pst2 = pss.tile([H, TPC], BF16, tag="ps", name="ps_dprT")
    nc.tensor.transpose(pst2[:], dpr_b[:], identb[:TPC, :TPC])
    dprT = big.tile([H, TPC], BF16, tag="dprT", name="dprT")
    nc.vector.tensor_copy(dprT[:], pst2[:])

    psd = pss.tile([TPC, H2], F32, tag="ps", name="ps_dz")
    _mm(nc, psd[:], dprT[:], m2wb[:], True, True)
    dz_b = big.tile([TPC, H2], BF16, tag="dz_b", name="dz_b")
    nc.vector.tensor_mul(dz_b[:], psd[:], sp[:])

    # grads: g1 [97, 192] = kp_aug^T dz ; g2 [97, 2, 96] = h_aug^T dpr
    psg1 = pss.tile([H + 1, H2], F32, tag="ps", name="ps_g1")
    _mm(nc, psg1[:], kp_b[:], dz_b[:], True, True)
    psg2 = pss.tile([H + 1, 2, H], F32, tag="ps", name="ps_g2")
    for m in range(2):
        _mm(nc, psg2[:, m, :], h_tm[:, m * (H + 1):(m + 1) * (H + 1)],
            dpr_b[:], True, True)
    grads = big.tile([H + 1, 4 * H], BF16, tag="grads", name="grads")
    nc.vector.tensor_copy(grads[:, 0:H2], psg1[:])
    nc.vector.tensor_copy(grads[:, H2:4 * H],
                          psg2[:].rearrange("p m h -> p (m h)"))
    nc.scalar.dma_start(gr_in[:, :], grads[:])
    nc.gpsimd.collective_compute(
        "AllReduce", OP.add,
        replica_groups=[list(range(NC))],
        ins=[gr_in[:, :].opt()],
        outs=[gr_out[:, :].opt()],
    )

    # q2 = l2norm over features (token-major) then transpose
    sqq = work.tile([TPC, H], F32, tag="sqq", name="sqq")
    nc.vector.tensor_mul(sqq[:], q2r[:], q2r[:])
    ssum = work.tile([TPC, 1], F32, tag="ssum", name="ssum")
    nc.vector.reduce_sum(ssum[:], sqq[:], axis=mybir.AxisListType.X)
    rs2 = work.tile([TPC, 1], F32, tag="rs2", name="rs2")
    _rsqrt(nc, work, "l2t", ssum[:], rs2[:])
    q2n = work.tile([TPC, H], BF16, tag="q2n", name="q2n")
    nc.vector.tensor_scalar_mul(q2n[:], q2r[:], rs2[:])
    ps_qt = pss.tile([H, TPC], BF16, tag="ps", name="ps_q2T")
    nc.tensor.transpose(ps_qt[:], q2n[:], identb[:TPC, :TPC])
    q2T = big.tile([H + 1, TPC], BF16, tag="q2T", name="q2T")
    nc.vector.tensor_copy(q2T[0:H, :], ps_qt[:])
    nc.vector.memset(q2T[H:H + 1, :], 1.0)

    # ============ F4: finalize ============
    grd = big.tile([H + 1, 4 * H], BF16, tag="grd", name="grd")
    nc.scalar.dma_start(grd[:], gr_out[:, :])
    THP = THETA * 2.0 / (TQ * H)

    if cfg.get("debug"):
        nc.sync.dma_start(dd["d_arin"][:], ar_in[:, :])
        nc.sync.dma_start(dd["d_rs"][:], ar_out[:, :])
        nc.sync.dma_start(dd["d_grads"][:], gr_out[:, :])
        nc.sync.dma_start(dd["d_xff"][:], xffT[:])
        dq2 = work.tile([H, TPC], F32, tag="dq2", name="dq2")
        nc.vector.tensor_copy(dq2[:], q2T[:])
        nc.sync.dma_start(dd["d_q2T"][:], dq2[:])
        dh = work.tile([TPC, 194], F32, tag="dh", name="dh")
        nc.vector.tensor_copy(dh[:], h_tm[:])
        nc.sync.dma_start(dd["d_h"][:], dh[:])
        ddz = work.tile([TPC, H2], F32, tag="ddz", name="ddz")
        nc.vector.tensor_copy(ddz[:], dz_b[:])
        nc.sync.dma_start(dd["d_dz"][:], ddz[:])

    nm1T = big.tile([H + 1, H2], F32, tag="nm1T", name="nm1T")
    tg1 = work.tile([H + 1, H2], F32, tag="tg1", name="tg1")
    nc.vector.tensor_scalar_mul(tg1[:], grd[:, 0:H2], THP)
    nc.vector.tensor_scalar_mul(nm1T[:], m1T[:], ALPHA)
    nc.vector.tensor_sub(nm1T[:], nm1T[:], tg1[:])
    nm1Tb = big.tile([H + 1, H2], BF16, tag="nm1Tb", name="nm1Tb")
    nc.vector.tensor_copy(nm1Tb[:], nm1T[:])
    if cfg.get("debug"):
        nc.sync.dma_start(dd["d_nm1T"][:], nm1T[0:H, :])

    nm2T = big.tile([H + 1, 2, H], BF16, tag="nm2T", name="nm2T")
    tg2 = work.tile([H + 1, 2, H], F32, tag="tg2", name="tg2")
    nc.vector.tensor_scalar_mul(tg2[:].rearrange("p m h -> p (m h)"),
                                grd[:, H2:4 * H], THP)
    tg2b = work.tile([H + 1, 2, H], F32, tag="tg2b", name="tg2b")
    nc.vector.tensor_scalar_mul(tg2b[:].rearrange("p m h -> p (m h)"),
                                m2T[:].rearrange("p m h -> p (m h)"), ALPHA)
    nc.vector.tensor_sub(nm2T[:].rearrange("p m h -> p (m h)"),
                         tg2b[:].rearrange("p m h -> p (m h)"),
                         tg2[:].rearrange("p m h -> p (m h)"))
    # chunk-1 bias row must not double-count nm2b
    nc.vector.memset(nm2T[H:H + 1, 1, :], 0.0)

    # retrieve with updated memory for own 96 tokens (biases ride the
    # augmented ones rows)
    uu = []
    for m in range(2):
        psu = pss.tile([H, TPC], F32, tag="ps", name="ps_u")
        _mm(nc, psu[:], nm1Tb[:, m * H:(m + 1) * H], q2T[:], True, True)
        thu = work.tile([H, TPC], F32, tag="thu", name="thu", bufs=2)
        nc.scalar.activation(thu[:], psu[:], AF.Tanh, bias=zb[:H, :], scale=0.5)
        nc.vector.tensor_scalar(thu[:], thu[:], 0.5, 0.5, OP.mult, OP.add)
        u_b = work.tile([H + 1, TPC], BF16, tag="u_b", name="u_b", bufs=2)
        nc.vector.tensor_mul(u_b[0:H, :], psu[:], thu[:])
        nc.vector.memset(u_b[H:H + 1, :], 1.0)
        uu.append(u_b)
    psy = pss.tile([H, TPC], F32, tag="ps", name="ps_y")
    _mm(nc, psy[:], nm2T[:, 0, :], uu[0][:], True, False)
    _mm(nc, psy[:], nm2T[:, 1, :], uu[1][:], False, True)
    thy = work.tile([H, TPC], F32, tag="thy", name="thy")
    nc.scalar.activation(thy[:], psy[:], AF.Tanh, bias=zb[:H, :], scale=0.5)
    nc.vector.tensor_scalar(thy[:], thy[:], 0.5, 0.5, OP.mult, OP.add)
    ot = work.tile([H, TPC], F32, tag="ot", name="ot")
    nc.vector.tensor_mul(ot[:], xffT[:], thy[:])

    out3 = dd["out_d"][:].rearrange("h (b s) -> h b s", b=B)
    ot_b = work.tile([H, B, NG * 3], F32, tag="ot_b", name="ot_b")
    nc.vector.tensor_copy(ot_b[:],
                          ot[:].rearrange("h (g sp b) -> h b (g sp)", g=NG, sp=3))
    for g in range(NG):
        nc.scalar.dma_start(
            out3[:, :, ds(24 * g + pid * 3, 3)],
            ot_b[:, :, 3 * g:3 * g + 3])

    stack.close()


def _layernorm(nc, pss, work, src_ap, dst_ap, w_ap, b_ap, ones_col, ones_row, eps1):
    """dst = LN(src) * w + b over the feature (partition) axis; [96, T] APs."""
    T = src_ap.shape[-1]
    ps_s = pss.tile([1, T], F32, tag="ps", name="ps_lns")
    _mm(nc, ps_s[:], ones_col[:H, :], src_ap, True, True)
    mean = work.tile([1, T], F32, tag="ln_mean", name="ln_mean")
    nc.vector.tensor_scalar_mul(mean[:], ps_s[:], 1.0 / H)
    sq = work.tile([H, T], F32, tag="ln_sq", name="ln_sq")
    nc.vector.tensor_mul(sq[:], src_ap, src_ap)
    ps_q = pss.tile([1, T], F32, tag="ps", name="ps_lnq")
    _mm(nc, ps_q[:], ones_col[:H, :], sq[:], True, True)
    var = work.tile([1, T], F32, tag="ln_var", name="ln_var")
    nc.scalar.activation(var[:], ps_q[:], AF.Identity, bias=eps1[:], scale=1.0 / H)
    m2t = work.tile([1, T], F32, tag="ln_m2", name="ln_m2")
    nc.vector.tensor_mul(m2t[:], mean[:], mean[:])
    nc.vector.tensor_sub(var[:], var[:], m2t[:])
    rstd = work.tile([1, T], F32, tag="ln_rstd", name="ln_rstd")
    _rsqrt(nc, work, "ln", var[:], rstd[:])
    nmr = work.tile([1, T], F32, tag="ln_nmr", name="ln_nmr")
    nc.vector.tensor_mul(nmr[:], mean[:], rstd[:])
    nc.vector.tensor_scalar_mul(nmr[:], nmr[:], -1.0)
    ps_a = pss.tile([H, T], F32, tag="ps", name="ps_lna")
    _mm(nc, ps_a[:], ones_row[:], rstd[:], True, True)
    ps_c = pss.tile([H, T], F32, tag="ps", name="ps_lnc")
    _mm(nc, ps_c[:], ones_row[:], nmr[:], True, True)
    t1 = work.tile([H, T], F32, tag="ln_t1", name="ln_t1")
    nc.vector.tensor_mul(t1[:], src_ap, ps_a[:])
    nc.vector.tensor_add(t1[:], t1[:], ps_c[:])
    nc.vector.tensor_scalar(dst_ap, t1[:], w_ap[:], b_ap[:], OP.mult, OP.add)


def prep_inmaps(inputs, cfg=None):
    cfg = cfg or CFG
    f32 = np.float32
    bf16 = ml_dtypes.bfloat16

    def T(a):
        return np.ascontiguousarray(np.asarray(a, f32).T)

    x = np.asarray(inputs["x"], f32)
    ipw = np.asarray(inputs["in_proj_w"], f32)   # [288, 96]
    ipb = np.asarray(inputs["in_proj_b"], f32)   # [288]
    sc = 1.0 / math.sqrt(HD)
    qw_part = ipw[0:H] * sc
    qb_part = ipb[0:H] * sc
    kw_part = ipw[H:2 * H]
    kb_part = ipb[H:2 * H]
    vw_part = ipw[2 * H:3 * H]
    vb_part = ipb[2 * H:3 * H]

    ipqT = np.ascontiguousarray(qw_part.T.reshape(H, NH, HD))
    ipkT = np.ascontiguousarray(kw_part.T.reshape(H, NH, HD))
    ipqb = np.ascontiguousarray(qb_part.reshape(NH, HD).T.reshape(HD, NH, 1))
    ipkb = np.ascontiguousarray(kb_part.reshape(NH, HD).T.reshape(HD, NH, 1))

    opw = np.asarray(inputs["out_proj_w"], f32)
    opT = np.ascontiguousarray(opw.T.reshape(NH, HD, H).transpose(1, 0, 2))

    f1b = np.asarray(inputs["ff1_b"], f32).reshape(FF // 128, 128, 1)
    f1b = np.ascontiguousarray(f1b.transpose(1, 0, 2))
    f2T = T(inputs["ff2_w"])
    f2T = np.ascontiguousarray(f2T.reshape(FF // 128, 128, H).transpose(1, 0, 2))

    m1w = np.asarray(inputs["m1_w"], f32)        # [192, 96]
    m1bv = np.asarray(inputs["m1_b"], f32)       # [192]
    m2w = np.asarray(inputs["m2_w"], f32)        # [96, 192]
    m2bv = np.asarray(inputs["m2_b"], f32)       # [96]
    kw = np.asarray(inputs["k_w"], f32)
    kb = np.asarray(inputs["k_b"], f32)
    vw = np.asarray(inputs["v_w"], f32)
    vb = np.asarray(inputs["v_b"], f32)
    qw = np.asarray(inputs["q_w"], f32)
    qbv = np.asarray(inputs["q_b"], f32)

    m1b = np.ascontiguousarray(m1bv.reshape(2, H, 1).transpose(1, 0, 2))
    m1T_aug = np.concatenate([T(m1w), m1bv.reshape(1, H2)], 0)        # [97, 192]
    m2T3 = np.ascontiguousarray(T(m2w).reshape(2, H, H).transpose(1, 0, 2))
    m2T_aug = np.zeros((H + 1, 2, H), f32)
    m2T_aug[0:H] = m2T3
    m2T_aug[H, 0] = m2bv                                              # chunk-0 bias row

    # Wcat [97, 481]: z | kp(+ones col) | vp' | q2
    wcat = np.zeros((H + 1, WCAT), f32)
    m1kw = m1w @ kw                               # [192, 96]
    wcat[0:H, 0:H2] = m1kw.T
    wcat[H, 0:H2] = m1w @ kb + m1bv
    wcat[0:H, H2:H2 + H] = kw.T
    wcat[H, H2:H2 + H] = kb
    wcat[H, H2 + H] = 1.0                         # ones column for g1 bias row
    wcat[0:H, H2 + H + 1:H2 + 2 * H + 1] = vw.T
    wcat[H, H2 + H + 1:H2 + 2 * H + 1] = vb - m2bv
    wcat[0:H, H2 + 2 * H + 1:WCAT] = qw.T
    wcat[H, H2 + 2 * H + 1:WCAT] = qbv

    fwT = np.ascontiguousarray(np.asarray(inputs["final_w"], f32).T)
    fb = np.asarray(inputs["final_b"], f32).reshape(S, H)

    col = lambda k: np.ascontiguousarray(np.asarray(inputs[k], f32).reshape(-1, 1))
    xTf = T(x.reshape(TQ, H))
    base = dict(
        xT=xTf, xTb=xTf.astype(bf16),
        pmT=T(inputs["persistent_memory"]),
        qwTb=T(qw).astype(bf16), qb=col("q_b"),
        ipqT=ipqT.astype(bf16), ipkT=ipkT.astype(bf16),
        ipvT=np.ascontiguousarray(vw_part.T).astype(bf16),
        ipqb=ipqb, ipkb=ipkb,
        ipvb=np.ascontiguousarray(vb_part.reshape(1, H)),
        opT=opT.astype(bf16), opb=col("out_proj_b"),
        ln1w=col("ln1_w"), ln1b=col("ln1_b"),
        ln2w=col("ln2_w"), ln2b=col("ln2_b"),
        f1T=T(inputs["ff1_w"]).astype(bf16), f1b=f1b,
        f1bh=np.ascontiguousarray(f1b * 0.5),
        f2T=f2T.astype(bf16), f2b=col("ff2_b"),
        m1Tb=m1T_aug.astype(bf16), m1T=m1T_aug,
        m1b=m1b,
        m2Tb=m2T_aug.astype(bf16), m2T=m2T_aug,
        m2b=col("m2_b"),
        m2wb=np.ascontiguousarray(m2w).astype(bf16),
        wcat=wcat.astype(bf16),
    )
    in_maps = []
    for c in range(NC):
        m = dict(base)
        shard = fwT[c * DK:(c + 1) * DK]                     # [(l h), 9216]
        shard_pad = np.concatenate([shard, np.zeros((64, DOUT), f32)], 0)
        packed = shard_pad.reshape(20, 128, NOC, OC).transpose(2, 1, 0, 3)
        m["WTc"] = np.ascontiguousarray(packed.astype(bf16))
        # fbs: final_b rows for this core's tokens in (g, sp, b) order
        ss = np.array([24 * g + 3 * c + d for g in range(NG) for d in range(3)])
        m["fbs"] = np.ascontiguousarray(
            np.repeat(fb[ss], B, axis=0))                    # [96, 96]
        in_maps.append(m)
    return in_maps


def get_nc(cfg=None):
    cfg = cfg or CFG
    key = tuple(sorted((k, str(v)) for k, v in cfg.items()))
    if key not in _CACHE:
        _CACHE[key] = build(cfg)
    return _CACHE[key]


def assemble(results):
    """Gather per-core output column slices into the full [B, S, H] output."""
    full = np.zeros((H, TQ), np.float32)
    for c in range(NC):
        outc = results[c]["outf"]                            # [96, 768]
        cols = np.array([b * S + 24 * g + 3 * c + d
                         for g in range(NG) for d in range(3) for b in range(B)])
        full[:, cols] = outc[:, cols]
    return np.ascontiguousarray(full.T).reshape(B, S, H)


def kernel(**inputs):
    nc = get_nc()
    in_maps = prep_inmaps(inputs)
    res = bass_utils.run_bass_kernel_spmd(
        nc, in_maps, core_ids=list(range(NC)), trace=False
    )
    return assemble(res.results)


if __name__ == "__main__":
    print("building...")
    get_nc()
    print("built")
